# revision 6
# baseline (speedup 1.0000x reference)
"""Multi-head attention (B=2, S=2048, D=1024, H=16, causal) on 8 TRN2 NeuronCores.

Sharding: 8 shards = 2 batches x 4 head-groups (4 heads each). Each core:
  - projects q/k/v for its batch through its head-group's weight slices
    (qhT/khT computed transposed: [head_dim, tok], vh natural: [tok, head_dim])
  - computes causal attention per head in the transposed layout
    ST[k_tok, q_tok] = Kh @ Qh^T, exp (no max-subtraction needed: logits are
    O(1) and masked entries are simply never computed), row of ones appended
    to the AV stationary yields softmax denominators for free,
  - normalizes, writes attn^T per head (only causal blocks; rest stays 0),
  - computes its partial output projection out_part = (attn @ vh) @ Wo_slice.
Host: shards/transposes inputs, sums the 4 partial outputs per batch, adds
biases that are structurally awkward on device (bo), and transposes attn back.

All matmuls run in float32r (TRN2 full-rate fp32 mode, ~1.6e-4 rel precision).
"""
import sys

sys.path.insert(0, "/opt/trn_rl_repo")

import numpy as np
from contextlib import ExitStack

import concourse.bass as bass
import concourse.tile as tile
from concourse import bacc, mybir
from concourse import bass_utils

F32 = mybir.dt.float32
F32R = mybir.dt.float32r

B, S, DM, NHEADS = 2, 2048, 1024, 16
NH = 4            # heads per core
DH = DM // NHEADS  # 64
HD = NH * DH      # 256 head dims per core
KT = DM // 128    # 8 contraction tiles for projections
NCHK = S // 512   # 4 q-chunks
NTOK = S // 128   # 16 token tiles / k-blocks
SCALE = 1.0 / np.sqrt(np.float32(DH))

_CACHE = {}


def _build_program(schedule, nmask, trace_scopes=False):
    """schedule: per chunk c, list of (kb, mask_idx|None). Same for all cores."""
    nc = bacc.Bacc("TRN2", target_bir_lowering=False, debug=False)

    qT_d = nc.dram_tensor("qT", [DM, S], F32R, kind="ExternalInput").ap()
    kT_d = nc.dram_tensor("kT", [DM, S], F32R, kind="ExternalInput").ap()
    vT_d = nc.dram_tensor("vT", [DM, S], F32R, kind="ExternalInput").ap()
    wq_d = nc.dram_tensor("wq", [DM, HD], F32R, kind="ExternalInput").ap()
    wk_d = nc.dram_tensor("wk", [DM, HD], F32R, kind="ExternalInput").ap()
    wv_d = nc.dram_tensor("wv", [DM, HD], F32R, kind="ExternalInput").ap()
    wo_d = nc.dram_tensor("wo", [HD, DM], F32R, kind="ExternalInput").ap()
    bq_d = nc.dram_tensor("bq", [HD, 1], F32, kind="ExternalInput").ap()
    bk_d = nc.dram_tensor("bk", [HD, 1], F32, kind="ExternalInput").ap()
    bv_d = nc.dram_tensor("bv", [1, HD], F32, kind="ExternalInput").ap()
    mt_d = None
    if nmask:
        mt_d = nc.dram_tensor("mt", [nmask, 128, 512], F32R, kind="ExternalInput").ap()

    attnT_d = nc.dram_tensor("attnT", [NH, S, S], F32, kind="ExternalOutput").ap()
    outp_d = nc.dram_tensor("outp", [S, DM], F32, kind="ExternalOutput").ap()

    with tile.TileContext(nc) as tc, ExitStack() as ctx:
        wpool = ctx.enter_context(tc.tile_pool(name="w", bufs=1))
        stream = ctx.enter_context(tc.tile_pool(name="stream", bufs=2))
        stexp = ctx.enter_context(tc.tile_pool(name="stexp", bufs=17))
        misc = ctx.enter_context(tc.tile_pool(name="misc", bufs=2))
        psA = ctx.enter_context(tc.tile_pool(name="psA", bufs=2, space="PSUM"))
        psS = ctx.enter_context(tc.tile_pool(name="psS", bufs=3, space="PSUM"))
        psO = ctx.enter_context(tc.tile_pool(name="psO", bufs=2, space="PSUM"))

        # ---- persistent constants -------------------------------------------------
        wq_s, wk_s, wv_s = [], [], []
        for name, lst, src in (("wq", wq_s, wq_d), ("wk", wk_s, wk_d),
                               ("wv", wv_s, wv_d)):
            for kt in range(KT):
                t = wpool.tile([128, HD], F32R, tag=f"{name}_{kt}")
                nc.sync.dma_start(t[:], src[kt * 128:(kt + 1) * 128, :])
                lst.append(t)
        wo_s = []
        for m in range(2):
            t = wpool.tile([128, DM], F32R, tag=f"wo{m}")
            nc.sync.dma_start(t[:], wo_d[m * 128:(m + 1) * 128, :])
            wo_s.append(t)
        bqt, bkt = [], []
        for m in range(2):
            t = wpool.tile([128, 1], F32, tag=f"bq{m}")
            nc.sync.dma_start(t[:], bq_d[m * 128:(m + 1) * 128, :])
            bqt.append(t)
            t = wpool.tile([128, 1], F32, tag=f"bk{m}")
            nc.sync.dma_start(t[:], bk_d[m * 128:(m + 1) * 128, :])
            bkt.append(t)
        bv_row = wpool.tile([1, HD], F32, tag="bvrow")
        nc.sync.dma_start(bv_row[:], bv_d)
        bv_bc = wpool.tile([128, HD], F32, tag="bvbc")
        nc.gpsimd.partition_broadcast(bv_bc[:], bv_row[:])

        mt_s = []
        for i in range(nmask):
            t = wpool.tile([128, 512], F32R, tag=f"mt{i}")
            nc.sync.dma_start(t[:], mt_d[i])
            mt_s.append(t)

        ones_f32 = wpool.tile([128, 1], F32, tag="ones")
        nc.vector.memset(ones_f32[:], 1.0)

        qhT = [wpool.tile([128, S], F32R, tag=f"qhT{m}", name=f"qhT{m}") for m in range(2)]
        khT = [wpool.tile([128, S], F32R, tag=f"khT{m}", name=f"khT{m}") for m in range(2)]
        aoT = [wpool.tile([128, S], F32R, tag=f"aoT{m}", name=f"aoT{m}") for m in range(2)]
        vh_pack = [wpool.tile([128, NTOK * (DH + 1)], F32R, tag=f"vhp{h}",
                              name=f"vhp{h}") for h in range(NH)]

        def vh_ones(h, tt):
            return vh_pack[h][:, tt * (DH + 1):(tt + 1) * (DH + 1)]

        for h in range(NH):
            for tt in range(NTOK):
                nc.vector.tensor_copy(vh_ones(h, tt)[:, DH:DH + 1], ones_f32[:])

        # ---- phase 2a: q/k projections (transposed: [head_dim, tok]) -------------
        for hc in range(8):
            cs = slice(hc * 256, (hc + 1) * 256)
            qt = []
            for kt in range(KT):
                t = stream.tile([128, 256], F32R, tag=f"s{kt}", name=f"qt{kt}")
                nc.sync.dma_start(t[:], qT_d[kt * 128:(kt + 1) * 128, cs])
                qt.append(t)
            kt_tiles = []
            for kt in range(KT):
                t = stream.tile([128, 256], F32R, tag=f"t{kt}", name=f"ktt{kt}")
                nc.sync.dma_start(t[:], kT_d[kt * 128:(kt + 1) * 128, cs])
                kt_tiles.append(t)
            for m in range(2):
                p = psA.tile([128, 256], F32, tag="psA")
                for kt in range(KT):
                    nc.tensor.matmul(p[:], wq_s[kt][:, m * 128:(m + 1) * 128],
                                     qt[kt][:], start=(kt == 0), stop=(kt == KT - 1))
                nc.scalar.activation(qhT[m][:, cs], p[:],
                                     mybir.ActivationFunctionType.Identity,
                                     bias=bqt[m][:])
                p = psA.tile([128, 256], F32, tag="psA")
                for kt in range(KT):
                    nc.tensor.matmul(p[:], wk_s[kt][:, m * 128:(m + 1) * 128],
                                     kt_tiles[kt][:], start=(kt == 0), stop=(kt == KT - 1))
                nc.scalar.activation(khT[m][:, cs], p[:],
                                     mybir.ActivationFunctionType.Identity,
                                     bias=bkt[m][:])

        # ---- phase 2b: v projection (natural: [tok, head_dim]) -------------------
        for hc in range(8):
            cs = slice(hc * 256, (hc + 1) * 256)
            vt = []
            for kt in range(KT):
                t = stream.tile([128, 256], F32R, tag=f"s{kt}", name=f"vt{kt}")
                nc.sync.dma_start(t[:], vT_d[kt * 128:(kt + 1) * 128, cs])
                vt.append(t)
            for tl in range(2):
                tt = hc * 2 + tl
                p = psA.tile([128, HD], F32, tag="psA")
                for kt in range(KT):
                    nc.tensor.matmul(p[:], vt[kt][:, tl * 128:(tl + 1) * 128],
                                     wv_s[kt][:], start=(kt == 0), stop=(kt == KT - 1))
                pb = misc.tile([128, HD], F32, tag="vbias")
                nc.vector.tensor_add(pb[:], p[:], bv_bc[:])
                for h in range(NH):
                    nc.vector.tensor_copy(vh_ones(h, tt)[:, 0:DH],
                                          pb[:, h * DH:(h + 1) * DH])

        # ---- phase 3: attention ---------------------------------------------------
        for h in range(NH):
            m, po = h // 2, (h % 2) * 64
            for c in range(NCHK):
                blocks = schedule[c]
                cs = slice(c * 512, (c + 1) * 512)
                pOut = psO.tile([DH + 1, 512], F32, tag="psO")
                st_tiles = []
                for i, (kb, mi) in enumerate(blocks):
                    pS = psS.tile([128, 512], F32, tag="psS")
                    nc.tensor.matmul(pS[:],
                                     khT[m][po:po + 64, kb * 128:(kb + 1) * 128],
                                     qhT[m][po:po + 64, cs],
                                     start=True, stop=True)
                    st = stexp.tile([128, 512], F32R, tag="st")
                    nc.scalar.activation(st[:], pS[:],
                                         mybir.ActivationFunctionType.Exp,
                                         scale=float(SCALE))
                    if mi is not None:
                        nc.vector.tensor_mul(st[:], st[:], mt_s[mi][:])
                    nc.tensor.matmul(pOut[:], vh_ones(h, kb)[:], st[:],
                                     start=(i == 0), stop=(i == len(blocks) - 1))
                    st_tiles.append((kb, st))
                sumrow = misc.tile([128, 512], F32, tag="sumrow")
                nc.scalar.activation(sumrow[64:65, :], pOut[DH:DH + 1, :],
                                     mybir.ActivationFunctionType.Copy)
                row0 = misc.tile([1, 512], F32, tag="row0")
                nc.sync.dma_start(row0[:], sumrow[64:65, :])
                recip0 = misc.tile([1, 512], F32R, tag="recip0")
                with nc.allow_low_precision(reason="f32r recip for normalize"):
                    nc.vector.reciprocal(recip0[:], row0[:])
                recip_bc = misc.tile([128, 512], F32R, tag="recipbc")
                nc.gpsimd.partition_broadcast(recip_bc[:], recip0[:])
                nc.vector.tensor_mul(aoT[m][po:po + 64, cs], pOut[0:DH, :],
                                     recip_bc[0:64, :])
                for kb, st in st_tiles:
                    nc.vector.tensor_mul(st[:], st[:], recip_bc[:])
                    nc.sync.dma_start(
                        attnT_d[h, kb * 128:(kb + 1) * 128, cs].bitcast(F32R), st[:])

        # ---- phase 4: output projection ------------------------------------------
        for tt in range(NTOK):
            for nn in range(2):
                p = psA.tile([128, 512], F32, tag="psA")
                for m in range(2):
                    nc.tensor.matmul(p[:], aoT[m][:, tt * 128:(tt + 1) * 128],
                                     wo_s[m][:, nn * 512:(nn + 1) * 512],
                                     start=(m == 0), stop=(m == 1))
                ev = misc.tile([128, 512], F32, tag="oev")
                nc.vector.tensor_copy(ev[:], p[:])
                nc.sync.dma_start(outp_d[tt * 128:(tt + 1) * 128,
                                         nn * 512:(nn + 1) * 512], ev[:])

    nc.compile()
    return nc


def _mask_schedule(mask):
    """Classify (k-block 128) x (q-chunk 512) tiles from mask (S,S), 1=masked.
    Returns (schedule, mask_tiles): schedule[c] = [(kb, mask_idx|None)...]."""
    masked = mask >= 0.5
    schedule = []
    tiles = []
    tile_index = {}
    for c in range(NCHK):
        blocks = []
        for kb in range(NTOK):
            sub = masked[c * 512:(c + 1) * 512, kb * 128:(kb + 1) * 128]  # (q, k)
            if sub.all():
                continue
            if not sub.any():
                blocks.append((kb, None))
                continue
            t = np.ascontiguousarray((~sub).T.astype(np.float32))  # (k128, q512)
            key = t.tobytes()
            if key not in tile_index:
                tile_index[key] = len(tiles)
                tiles.append(t)
            blocks.append((kb, tile_index[key]))
        schedule.append(blocks)
    return schedule, tiles


def kernel(q, k, v, mask, Wq, bq, Wk, bk, Wv, bv, Wo, bo, _trace=False):
    q = np.asarray(q, np.float32)
    k = np.asarray(k, np.float32)
    v = np.asarray(v, np.float32)
    mask2d = np.asarray(mask, np.float32).reshape(S, S)
    Wq, Wk, Wv, Wo = (np.asarray(x, np.float32) for x in (Wq, Wk, Wv, Wo))
    bq, bk, bv, bo = (np.asarray(x, np.float32) for x in (bq, bk, bv, bo))

    schedule, mtiles = _mask_schedule(mask2d)
    key = (tuple(tuple(b) for b in schedule), len(mtiles))
    if key not in _CACHE:
        _CACHE[key] = _build_program(schedule, len(mtiles))
    nc = _CACHE[key]

    mt_arr = np.stack(mtiles) if mtiles else None
    in_maps = []
    for c in range(8):
        b, g = c // 4, c % 4
        hs = slice(g * HD, (g + 1) * HD)
        im = {
            "qT": np.ascontiguousarray(q[b].T),
            "kT": np.ascontiguousarray(k[b].T),
            "vT": np.ascontiguousarray(v[b].T),
            "wq": np.ascontiguousarray(Wq[:, hs]),
            "wk": np.ascontiguousarray(Wk[:, hs]),
            "wv": np.ascontiguousarray(Wv[:, hs]),
            "wo": np.ascontiguousarray(Wo[hs, :]),
            "bq": np.ascontiguousarray(bq[hs].reshape(HD, 1)),
            "bk": np.ascontiguousarray(bk[hs].reshape(HD, 1)),
            "bv": np.ascontiguousarray(bv[hs].reshape(1, HD)),
        }
        if mt_arr is not None:
            im["mt"] = mt_arr
        in_maps.append(im)

    res = bass_utils.run_bass_kernel_spmd(nc, in_maps, core_ids=list(range(8)),
                                          trace=_trace)

    out = np.zeros((B, S, DM), np.float32)
    attn = np.empty((B, NHEADS, S, S), np.float32)
    for c in range(8):
        b, g = c // 4, c % 4
        r = res.results[c]
        out[b] += r["outp"]
        at = r["attnT"]  # (NH, S, S) [h, k, q]
        for h in range(NH):
            attn[b, g * NH + h] = at[h].T
    out += bo
    if _trace:
        kernel.last_results = res
    return out, attn


# revision 7
# speedup vs baseline: 1.0777x; 1.0777x over previous
"""Multi-head attention (B=2, S=2048, D=1024, H=16, causal) on 8 TRN2 NeuronCores.

Sharding: 8 shards = 2 batches x 4 head-groups (4 heads each). Each core:
  - projects q/k/v for its batch through its head-group's weight slices
    (qhT/khT computed transposed: [head_dim, tok]; vh natural: [tok, head_dim])
  - computes causal attention per head in the transposed layout
    ST[k_tok, q_tok] = Kh @ Qh^T, exp (no max-subtraction: logits are O(1) and
    masked entries are never computed), a column of ones appended to the AV
    stationary yields the softmax denominators for free,
  - normalizes, writes attn^T per head (only causal blocks; the rest of the
    output buffer stays zero), in a DMA-contiguous blocked layout,
  - computes its partial output projection out_part = (attn @ vh) @ Wo_slice.
Host: shards/transposes inputs into blocked layouts, sums the 4 partial
outputs per batch, adds bo, and un-blocks/transposes attn back.

Matmul dtype: bf16 (full-rate PE) by default; float32r (half-rate, ~1.6e-4)
via KERNEL_F32R=1. PSUM accumulation is fp32 in both.
"""
import os
import sys

sys.path.insert(0, "/opt/trn_rl_repo")

import numpy as np
import ml_dtypes
from contextlib import ExitStack

import concourse.bass as bass
import concourse.tile as tile
from concourse import bacc, mybir
from concourse import bass_utils

F32 = mybir.dt.float32
F32R = mybir.dt.float32r
BF16 = mybir.dt.bfloat16

B, S, DM, NHEADS = 2, 2048, 1024, 16
NH = 4             # heads per core
DH = DM // NHEADS  # 64
HD = NH * DH       # 256 head dims per core
KT = DM // 128     # 8 contraction tiles for projections
NCHK = S // 512    # 4 q-chunks
NHC = S // 256     # 8 projection half-chunks
NTOK = S // 128    # 16 token tiles / k-blocks
SCALE = 1.0 / np.sqrt(np.float32(DH))

USE_F32R = os.environ.get("KERNEL_F32R", "0") == "1"

_CACHE = {}


def _build_program(schedule, nmask, use_f32r):
    """schedule: per chunk c, list of (kb, mask_idx|None). Same on all cores."""
    DT = F32R if use_f32r else BF16
    nc = bacc.Bacc("TRN2", target_bir_lowering=False, debug=False)

    # blocked inputs: [KT, NHC, 128, 256] so each DMA'd tile is contiguous
    qT_d = nc.dram_tensor("qT", [KT, NHC, 128, 256], DT, kind="ExternalInput").ap()
    kT_d = nc.dram_tensor("kT", [KT, NHC, 128, 256], DT, kind="ExternalInput").ap()
    vT_d = nc.dram_tensor("vT", [KT, NHC, 128, 256], DT, kind="ExternalInput").ap()
    wq_d = nc.dram_tensor("wq", [DM, HD], DT, kind="ExternalInput").ap()
    wk_d = nc.dram_tensor("wk", [DM, HD], DT, kind="ExternalInput").ap()
    wv_d = nc.dram_tensor("wv", [DM, HD], DT, kind="ExternalInput").ap()
    wo_d = nc.dram_tensor("wo", [HD, DM], DT, kind="ExternalInput").ap()
    bq_d = nc.dram_tensor("bq", [HD, 1], F32, kind="ExternalInput").ap()
    bk_d = nc.dram_tensor("bk", [HD, 1], F32, kind="ExternalInput").ap()
    bv_d = nc.dram_tensor("bv", [1, HD], F32, kind="ExternalInput").ap()
    mt_d = None
    if nmask:
        mt_d = nc.dram_tensor("mt", [nmask, 128, 512], DT, kind="ExternalInput").ap()

    # blocked outputs: every [128, 512] store is one contiguous region
    attnT_d = nc.dram_tensor("attnT", [NH, NTOK, NCHK, 128, 512], DT,
                             kind="ExternalOutput").ap()
    outp_d = nc.dram_tensor("outp", [NTOK, 2, 128, 512], F32,
                            kind="ExternalOutput").ap()

    with tile.TileContext(nc) as tc, ExitStack() as ctx:
        wpool = ctx.enter_context(tc.tile_pool(name="w", bufs=1))
        stream = ctx.enter_context(tc.tile_pool(name="stream", bufs=2))
        stexp = ctx.enter_context(tc.tile_pool(name="stexp", bufs=17))
        misc = ctx.enter_context(tc.tile_pool(name="misc", bufs=2))
        psA = ctx.enter_context(tc.tile_pool(name="psA", bufs=2, space="PSUM"))
        psS = ctx.enter_context(tc.tile_pool(name="psS", bufs=3, space="PSUM"))
        psO = ctx.enter_context(tc.tile_pool(name="psO", bufs=2, space="PSUM"))

        # ---- persistent constants ------------------------------------------------
        wq_s, wk_s, wv_s = [], [], []
        for name, lst, src in (("wq", wq_s, wq_d), ("wk", wk_s, wk_d),
                               ("wv", wv_s, wv_d)):
            for kt in range(KT):
                t = wpool.tile([128, HD], DT, tag=f"{name}_{kt}", name=f"{name}_{kt}")
                nc.sync.dma_start(t[:], src[kt * 128:(kt + 1) * 128, :])
                lst.append(t)
        wo_s = []
        for m in range(2):
            t = wpool.tile([128, DM], DT, tag=f"wo{m}", name=f"wo{m}")
            nc.sync.dma_start(t[:], wo_d[m * 128:(m + 1) * 128, :])
            wo_s.append(t)
        bqt, bkt = [], []
        for m in range(2):
            t = wpool.tile([128, 1], F32, tag=f"bq{m}", name=f"bq{m}")
            nc.sync.dma_start(t[:], bq_d[m * 128:(m + 1) * 128, :])
            bqt.append(t)
            t = wpool.tile([128, 1], F32, tag=f"bk{m}", name=f"bk{m}")
            nc.sync.dma_start(t[:], bk_d[m * 128:(m + 1) * 128, :])
            bkt.append(t)
        bv_row = wpool.tile([1, HD], F32, tag="bvrow")
        nc.sync.dma_start(bv_row[:], bv_d)
        bv_bc = wpool.tile([128, HD], F32, tag="bvbc")
        nc.gpsimd.partition_broadcast(bv_bc[:], bv_row[:])

        mt_s = []
        for i in range(nmask):
            t = wpool.tile([128, 512], DT, tag=f"mt{i}", name=f"mt{i}")
            nc.sync.dma_start(t[:], mt_d[i])
            mt_s.append(t)

        ones_f32 = wpool.tile([128, 1], F32, tag="ones")
        nc.vector.memset(ones_f32[:], 1.0)

        qhT = [wpool.tile([128, S], DT, tag=f"qhT{m}", name=f"qhT{m}")
               for m in range(2)]
        khT = [wpool.tile([128, S], DT, tag=f"khT{m}", name=f"khT{m}")
               for m in range(2)]
        aoT = [wpool.tile([128, S], DT, tag=f"aoT{m}", name=f"aoT{m}")
               for m in range(2)]
        vh_pack = [wpool.tile([128, NTOK * (DH + 1)], DT, tag=f"vhp{h}",
                              name=f"vhp{h}") for h in range(NH)]

        def vh_ones(h, tt):
            return vh_pack[h][:, tt * (DH + 1):(tt + 1) * (DH + 1)]

        for h in range(NH):
            for tt in range(NTOK):
                nc.vector.tensor_copy(vh_ones(h, tt)[:, DH:DH + 1], ones_f32[:])

        # ---- phase 2a: q/k projections (transposed: [head_dim, tok]) ------------
        for hc in range(NHC):
            cs = slice(hc * 256, (hc + 1) * 256)
            qt = []
            for kt in range(KT):
                t = stream.tile([128, 256], DT, tag=f"s{kt}", name=f"qt{kt}")
                nc.sync.dma_start(t[:], qT_d[kt, hc])
                qt.append(t)
            ktt = []
            for kt in range(KT):
                t = stream.tile([128, 256], DT, tag=f"t{kt}", name=f"ktt{kt}")
                nc.sync.dma_start(t[:], kT_d[kt, hc])
                ktt.append(t)
            for m in range(2):
                p = psA.tile([128, 256], F32, tag="psA", name="pq")
                for kt in range(KT):
                    nc.tensor.matmul(p[:], wq_s[kt][:, m * 128:(m + 1) * 128],
                                     qt[kt][:], start=(kt == 0), stop=(kt == KT - 1))
                nc.scalar.activation(qhT[m][:, cs], p[:],
                                     mybir.ActivationFunctionType.Identity,
                                     bias=bqt[m][:])
                p = psA.tile([128, 256], F32, tag="psA", name="pk")
                for kt in range(KT):
                    nc.tensor.matmul(p[:], wk_s[kt][:, m * 128:(m + 1) * 128],
                                     ktt[kt][:], start=(kt == 0), stop=(kt == KT - 1))
                nc.scalar.activation(khT[m][:, cs], p[:],
                                     mybir.ActivationFunctionType.Identity,
                                     bias=bkt[m][:])

        # ---- phase 2b: v projection (natural: [tok, head_dim]) ------------------
        for hc in range(NHC):
            vt = []
            for kt in range(KT):
                t = stream.tile([128, 256], DT, tag=f"s{kt}", name=f"vt{kt}")
                nc.sync.dma_start(t[:], vT_d[kt, hc])
                vt.append(t)
            for tl in range(2):
                tt = hc * 2 + tl
                p = psA.tile([128, HD], F32, tag="psA", name="pv")
                for kt in range(KT):
                    nc.tensor.matmul(p[:], vt[kt][:, tl * 128:(tl + 1) * 128],
                                     wv_s[kt][:], start=(kt == 0), stop=(kt == KT - 1))
                pb = misc.tile([128, HD], F32, tag="vbias", name="pb")
                nc.vector.tensor_add(pb[:], p[:], bv_bc[:])
                for h in range(NH):
                    nc.vector.tensor_copy(vh_ones(h, tt)[:, 0:DH],
                                          pb[:, h * DH:(h + 1) * DH])

        # ---- phase 3: attention --------------------------------------------------
        for h in range(NH):
            m, po = h // 2, (h % 2) * 64
            for c in range(NCHK):
                blocks = schedule[c]
                cs = slice(c * 512, (c + 1) * 512)
                pOut = psO.tile([DH + 1, 512], F32, tag="psO", name="pOut")
                st_tiles = []
                for i, (kb, mi) in enumerate(blocks):
                    pS = psS.tile([128, 512], F32, tag="psS", name="pS")
                    nc.tensor.matmul(pS[:],
                                     khT[m][po:po + 64, kb * 128:(kb + 1) * 128],
                                     qhT[m][po:po + 64, cs],
                                     start=True, stop=True)
                    st = stexp.tile([128, 512], DT, tag="st", name="st")
                    nc.scalar.activation(st[:], pS[:],
                                         mybir.ActivationFunctionType.Exp,
                                         scale=float(SCALE))
                    if mi is not None:
                        nc.vector.tensor_mul(st[:], st[:], mt_s[mi][:])
                    nc.tensor.matmul(pOut[:], vh_ones(h, kb)[:], st[:],
                                     start=(i == 0), stop=(i == len(blocks) - 1))
                    st_tiles.append((kb, st))
                sumrow = misc.tile([128, 512], F32, tag="sumrow", name="sumrow")
                nc.scalar.activation(sumrow[64:65, :], pOut[DH:DH + 1, :],
                                     mybir.ActivationFunctionType.Copy)
                row0 = misc.tile([1, 512], F32, tag="row0", name="row0")
                nc.sync.dma_start(row0[:], sumrow[64:65, :])
                recip0 = misc.tile([1, 512], F32, tag="recip0", name="recip0")
                nc.vector.reciprocal(recip0[:], row0[:])
                recip_bc = misc.tile([128, 512], F32, tag="recipbc", name="recipbc")
                nc.gpsimd.partition_broadcast(recip_bc[:], recip0[:])
                nc.vector.tensor_mul(aoT[m][po:po + 64, cs], pOut[0:DH, :],
                                     recip_bc[0:64, :])
                for kb, st in st_tiles:
                    nc.vector.tensor_mul(st[:], st[:], recip_bc[:])
                    nc.sync.dma_start(attnT_d[h, kb, c], st[:])

        # ---- phase 4: output projection -----------------------------------------
        for tt in range(NTOK):
            for nn in range(2):
                p = psA.tile([128, 512], F32, tag="psA", name="po")
                for m in range(2):
                    nc.tensor.matmul(p[:], aoT[m][:, tt * 128:(tt + 1) * 128],
                                     wo_s[m][:, nn * 512:(nn + 1) * 512],
                                     start=(m == 0), stop=(m == 1))
                ev = misc.tile([128, 512], F32, tag="oev", name="ev")
                nc.vector.tensor_copy(ev[:], p[:])
                nc.sync.dma_start(outp_d[tt, nn], ev[:])

    nc.compile()
    return nc


def _mask_schedule(mask):
    """Classify (k-block 128) x (q-chunk 512) tiles from mask (S,S), 1=masked."""
    masked = mask >= 0.5
    schedule = []
    tiles = []
    tile_index = {}
    for c in range(NCHK):
        blocks = []
        for kb in range(NTOK):
            sub = masked[c * 512:(c + 1) * 512, kb * 128:(kb + 1) * 128]  # (q, k)
            if sub.all():
                continue
            if not sub.any():
                blocks.append((kb, None))
                continue
            t = np.ascontiguousarray((~sub).T.astype(np.float32))  # (k128, q512)
            key = t.tobytes()
            if key not in tile_index:
                tile_index[key] = len(tiles)
                tiles.append(t)
            blocks.append((kb, tile_index[key]))
        schedule.append(blocks)
    return schedule, tiles


def _block_T(x):
    """(S, DM) -> transposed blocked [KT, NHC, 128, 256] contiguous."""
    # xT[d, s]: block [kt, hc, p, j] = x[hc*256+j, kt*128+p]
    return np.ascontiguousarray(
        x.T.reshape(KT, 128, NHC, 256).transpose(0, 2, 1, 3))


def kernel(q, k, v, mask, Wq, bq, Wk, bk, Wv, bv, Wo, bo, _trace=False):
    q = np.asarray(q, np.float32)
    k = np.asarray(k, np.float32)
    v = np.asarray(v, np.float32)
    mask2d = np.asarray(mask, np.float32).reshape(S, S)
    Wq, Wk, Wv, Wo = (np.asarray(x, np.float32) for x in (Wq, Wk, Wv, Wo))
    bq, bk, bv, bo = (np.asarray(x, np.float32) for x in (bq, bk, bv, bo))

    use_f32r = USE_F32R
    ndt = np.float32 if use_f32r else ml_dtypes.bfloat16

    schedule, mtiles = _mask_schedule(mask2d)
    key = (tuple(tuple(b) for b in schedule), len(mtiles), use_f32r)
    if key not in _CACHE:
        _CACHE[key] = _build_program(schedule, len(mtiles), use_f32r)
    nc = _CACHE[key]

    mt_arr = np.stack(mtiles).astype(ndt) if mtiles else None
    in_maps = []
    for c in range(8):
        b, g = c // 4, c % 4
        hs = slice(g * HD, (g + 1) * HD)
        im = {
            "qT": _block_T(q[b]).astype(ndt),
            "kT": _block_T(k[b]).astype(ndt),
            "vT": _block_T(v[b]).astype(ndt),
            "wq": np.ascontiguousarray(Wq[:, hs]).astype(ndt),
            "wk": np.ascontiguousarray(Wk[:, hs]).astype(ndt),
            "wv": np.ascontiguousarray(Wv[:, hs]).astype(ndt),
            "wo": np.ascontiguousarray(Wo[hs, :]).astype(ndt),
            "bq": np.ascontiguousarray(bq[hs].reshape(HD, 1)),
            "bk": np.ascontiguousarray(bk[hs].reshape(HD, 1)),
            "bv": np.ascontiguousarray(bv[hs].reshape(1, HD)),
        }
        if mt_arr is not None:
            im["mt"] = mt_arr
        in_maps.append(im)

    res = bass_utils.run_bass_kernel_spmd(nc, in_maps, core_ids=list(range(8)),
                                          trace=_trace)

    out = np.zeros((B, S, DM), np.float32)
    attn = np.empty((B, NHEADS, S, S), np.float32)
    for c in range(8):
        b, g = c // 4, c % 4
        r = res.results[c]
        op = np.asarray(r["outp"], np.float32)  # [NTOK, 2, 128, 512]
        out[b] += op.transpose(0, 2, 1, 3).reshape(S, DM)
        at = np.asarray(r["attnT"]).astype(np.float32)  # [NH, NTOK, NCHK, 128, 512]
        for h in range(NH):
            # blocked [kb, c, kl, qj] -> attn[q, k]: q=c*512+qj, k=kb*128+kl
            attn[b, g * NH + h] = at[h].transpose(1, 3, 0, 2).reshape(S, S)
    out += bo
    if _trace:
        kernel.last_results = res
    return out, attn


# revision 8
# speedup vs baseline: 1.4122x; 1.3103x over previous
"""Multi-head attention (B=2, S=2048, D=1024, H=16, causal) on 8 TRN2 NeuronCores.

Sharding: 8 shards = 2 batches x 4 head-groups (4 heads each). Each core:
  - projects q/k/v for its batch through its head-group's weight slices
    (qhT/khT computed transposed: [head_dim, tok]; vh natural: [tok, head_dim])
  - computes causal attention per head in the transposed layout
    ST[k_tok, q_tok] = Kh @ Qh^T, exp (no max-subtraction: logits are O(1) and
    masked entries are never computed), a column of ones appended to the AV
    stationary yields the softmax denominators for free,
  - normalizes, writes attn^T per head (only causal blocks; the rest of the
    output buffer stays zero), in a DMA-contiguous blocked layout,
  - computes its partial output projection out_part = (attn @ vh) @ Wo_slice.
Host: shards/transposes inputs into blocked layouts, sums the 4 partial
outputs per batch, adds bo, and un-blocks/transposes attn back.

Matmul dtype: bf16 (full-rate PE) by default; float32r (half-rate, ~1.6e-4)
via KERNEL_F32R=1. PSUM accumulation is fp32 in both.
"""
import os
import sys

sys.path.insert(0, "/opt/trn_rl_repo")

import numpy as np
import ml_dtypes
from contextlib import ExitStack

import concourse.bass as bass
import concourse.tile as tile
from concourse import bacc, mybir
from concourse import bass_utils

F32 = mybir.dt.float32
F32R = mybir.dt.float32r
BF16 = mybir.dt.bfloat16

B, S, DM, NHEADS = 2, 2048, 1024, 16
NH = 4             # heads per core
DH = DM // NHEADS  # 64
HD = NH * DH       # 256 head dims per core
KT = DM // 128     # 8 contraction tiles for projections
NCHK = S // 512    # 4 q-chunks
NHC = S // 256     # 8 projection half-chunks
NTOK = S // 128    # 16 token tiles / k-blocks
SCALE = 1.0 / np.sqrt(np.float32(DH))

USE_F32R = os.environ.get("KERNEL_F32R", "0") == "1"

_CACHE = {}


def _build_program(schedule, nmask, use_f32r):
    """schedule: per chunk c, list of (kb, mask_idx|None). Same on all cores."""
    DT = F32R if use_f32r else BF16
    nc = bacc.Bacc("TRN2", target_bir_lowering=False, debug=False)

    # blocked inputs: [KT, NHC, 128, 256] so each DMA'd tile is contiguous
    qT_d = nc.dram_tensor("qT", [KT, NHC, 128, 256], DT, kind="ExternalInput").ap()
    kT_d = nc.dram_tensor("kT", [KT, NHC, 128, 256], DT, kind="ExternalInput").ap()
    vT_d = nc.dram_tensor("vT", [KT, NHC, 128, 256], DT, kind="ExternalInput").ap()
    wq_d = nc.dram_tensor("wq", [DM, HD], DT, kind="ExternalInput").ap()
    wk_d = nc.dram_tensor("wk", [DM, HD], DT, kind="ExternalInput").ap()
    wv_d = nc.dram_tensor("wv", [DM, HD], DT, kind="ExternalInput").ap()
    wo_d = nc.dram_tensor("wo", [HD, DM], DT, kind="ExternalInput").ap()
    bq_d = nc.dram_tensor("bq", [HD, 1], F32, kind="ExternalInput").ap()
    bk_d = nc.dram_tensor("bk", [HD, 1], F32, kind="ExternalInput").ap()
    bv_d = nc.dram_tensor("bv", [1, HD], F32, kind="ExternalInput").ap()
    mt_d = None
    if nmask:
        mt_d = nc.dram_tensor("mt", [nmask, 128, 512], DT, kind="ExternalInput").ap()

    # blocked outputs: every [128, 512] store is one contiguous region
    attnT_d = nc.dram_tensor("attnT", [NH, NTOK, NCHK, 128, 512], DT,
                             kind="ExternalOutput").ap()
    outp_d = nc.dram_tensor("outp", [NTOK, 2, 128, 512], F32,
                            kind="ExternalOutput").ap()
    sums_d = nc.dram_tensor("sums", [NH, NCHK, 1, 512], F32,
                            kind="ExternalOutput").ap()

    with tile.TileContext(nc) as tc, ExitStack() as ctx:
        wpool = ctx.enter_context(tc.tile_pool(name="w", bufs=1))
        stream = ctx.enter_context(tc.tile_pool(name="stream", bufs=2))
        stexp = ctx.enter_context(tc.tile_pool(name="stexp", bufs=8))
        misc = ctx.enter_context(tc.tile_pool(name="misc", bufs=2))
        psA = ctx.enter_context(tc.tile_pool(name="psA", bufs=2, space="PSUM"))
        psS = ctx.enter_context(tc.tile_pool(name="psS", bufs=3, space="PSUM"))
        psO = ctx.enter_context(tc.tile_pool(name="psO", bufs=2, space="PSUM"))

        # ---- persistent constants ------------------------------------------------
        wq_s, wk_s, wv_s = [], [], []
        for name, lst, src in (("wq", wq_s, wq_d), ("wk", wk_s, wk_d),
                               ("wv", wv_s, wv_d)):
            for kt in range(KT):
                t = wpool.tile([128, HD], DT, tag=f"{name}_{kt}", name=f"{name}_{kt}")
                nc.sync.dma_start(t[:], src[kt * 128:(kt + 1) * 128, :])
                lst.append(t)
        wo_s = []
        for m in range(2):
            t = wpool.tile([128, DM], DT, tag=f"wo{m}", name=f"wo{m}")
            nc.sync.dma_start(t[:], wo_d[m * 128:(m + 1) * 128, :])
            wo_s.append(t)
        bqt, bkt = [], []
        for m in range(2):
            t = wpool.tile([128, 1], F32, tag=f"bq{m}", name=f"bq{m}")
            nc.sync.dma_start(t[:], bq_d[m * 128:(m + 1) * 128, :])
            bqt.append(t)
            t = wpool.tile([128, 1], F32, tag=f"bk{m}", name=f"bk{m}")
            nc.sync.dma_start(t[:], bk_d[m * 128:(m + 1) * 128, :])
            bkt.append(t)
        bv_row = wpool.tile([1, HD], F32, tag="bvrow")
        nc.sync.dma_start(bv_row[:], bv_d)
        bv_bc = wpool.tile([128, HD], F32, tag="bvbc")
        nc.gpsimd.partition_broadcast(bv_bc[:], bv_row[:])

        mt_s = []
        for i in range(nmask):
            t = wpool.tile([128, 512], DT, tag=f"mt{i}", name=f"mt{i}")
            nc.sync.dma_start(t[:], mt_d[i])
            mt_s.append(t)

        ones_f32 = wpool.tile([128, 1], F32, tag="ones")
        nc.vector.memset(ones_f32[:], 1.0)

        qhT = [wpool.tile([128, S], DT, tag=f"qhT{m}", name=f"qhT{m}")
               for m in range(2)]
        khT = [wpool.tile([128, S], DT, tag=f"khT{m}", name=f"khT{m}")
               for m in range(2)]
        aoT = [wpool.tile([128, S], DT, tag=f"aoT{m}", name=f"aoT{m}")
               for m in range(2)]
        vh_pack = [wpool.tile([128, NTOK * (DH + 1)], DT, tag=f"vhp{h}",
                              name=f"vhp{h}") for h in range(NH)]

        def vh_ones(h, tt):
            return vh_pack[h][:, tt * (DH + 1):(tt + 1) * (DH + 1)]

        for h in range(NH):
            for tt in range(NTOK):
                nc.vector.tensor_copy(vh_ones(h, tt)[:, DH:DH + 1], ones_f32[:])

        # ---- phase 2a: q/k projections (transposed: [head_dim, tok]) ------------
        for hc in range(NHC):
            cs = slice(hc * 256, (hc + 1) * 256)
            qt = []
            for kt in range(KT):
                t = stream.tile([128, 256], DT, tag=f"s{kt}", name=f"qt{kt}")
                nc.sync.dma_start(t[:], qT_d[kt, hc])
                qt.append(t)
            ktt = []
            for kt in range(KT):
                t = stream.tile([128, 256], DT, tag=f"t{kt}", name=f"ktt{kt}")
                nc.sync.dma_start(t[:], kT_d[kt, hc])
                ktt.append(t)
            for m in range(2):
                p = psA.tile([128, 256], F32, tag="psA", name="pq")
                for kt in range(KT):
                    nc.tensor.matmul(p[:], wq_s[kt][:, m * 128:(m + 1) * 128],
                                     qt[kt][:], start=(kt == 0), stop=(kt == KT - 1))
                nc.scalar.activation(qhT[m][:, cs], p[:],
                                     mybir.ActivationFunctionType.Identity,
                                     bias=bqt[m][:])
                p = psA.tile([128, 256], F32, tag="psA", name="pk")
                for kt in range(KT):
                    nc.tensor.matmul(p[:], wk_s[kt][:, m * 128:(m + 1) * 128],
                                     ktt[kt][:], start=(kt == 0), stop=(kt == KT - 1))
                nc.scalar.activation(khT[m][:, cs], p[:],
                                     mybir.ActivationFunctionType.Identity,
                                     bias=bkt[m][:])

        # ---- phase 2b: v projection (natural: [tok, head_dim]) ------------------
        for hc in range(NHC):
            vt = []
            for kt in range(KT):
                t = stream.tile([128, 256], DT, tag=f"s{kt}", name=f"vt{kt}")
                nc.sync.dma_start(t[:], vT_d[kt, hc])
                vt.append(t)
            for tl in range(2):
                tt = hc * 2 + tl
                p = psA.tile([128, HD], F32, tag="psA", name="pv")
                for kt in range(KT):
                    nc.tensor.matmul(p[:], vt[kt][:, tl * 128:(tl + 1) * 128],
                                     wv_s[kt][:], start=(kt == 0), stop=(kt == KT - 1))
                pb = misc.tile([128, HD], F32, tag="vbias", name="pb")
                nc.vector.tensor_add(pb[:], p[:], bv_bc[:])
                for h in range(NH):
                    nc.vector.tensor_copy(vh_ones(h, tt)[:, 0:DH],
                                          pb[:, h * DH:(h + 1) * DH])

        # ---- phase 3: attention --------------------------------------------------
        for h in range(NH):
            m, po = h // 2, (h % 2) * 64
            for c in range(NCHK):
                blocks = schedule[c]
                cs = slice(c * 512, (c + 1) * 512)
                pOut = psO.tile([DH + 1, 512], F32, tag="psO", name="pOut")
                for i, (kb, mi) in enumerate(blocks):
                    pS = psS.tile([128, 512], F32, tag="psS", name="pS")
                    nc.tensor.matmul(pS[:],
                                     khT[m][po:po + 64, kb * 128:(kb + 1) * 128],
                                     qhT[m][po:po + 64, cs],
                                     start=True, stop=True)
                    st = stexp.tile([128, 512], DT, tag="st", name="st")
                    nc.scalar.activation(st[:], pS[:],
                                         mybir.ActivationFunctionType.Exp,
                                         scale=float(SCALE))
                    if mi is not None:
                        nc.vector.tensor_mul(st[:], st[:], mt_s[mi][:])
                    nc.tensor.matmul(pOut[:], vh_ones(h, kb)[:], st[:],
                                     start=(i == 0), stop=(i == len(blocks) - 1))
                    nc.sync.dma_start(attnT_d[h, kb, c], st[:])
                sumrow = misc.tile([128, 512], F32, tag="sumrow", name="sumrow")
                nc.scalar.activation(sumrow[64:65, :], pOut[DH:DH + 1, :],
                                     mybir.ActivationFunctionType.Copy)
                nc.sync.dma_start(sums_d[h, c], sumrow[64:65, :])
                row0 = misc.tile([1, 512], F32, tag="row0", name="row0")
                nc.sync.dma_start(row0[:], sumrow[64:65, :])
                recip0 = misc.tile([1, 512], F32, tag="recip0", name="recip0")
                nc.vector.reciprocal_approx_fast(recip0[:], row0[:])
                recip_bc = misc.tile([64, 512], F32, tag="recipbc", name="recipbc")
                nc.gpsimd.partition_broadcast(recip_bc[:], recip0[:])
                nc.vector.tensor_mul(aoT[m][po:po + 64, cs], pOut[0:DH, :],
                                     recip_bc[:])

        # ---- phase 4: output projection -----------------------------------------
        for tt in range(NTOK):
            for nn in range(2):
                p = psA.tile([128, 512], F32, tag="psA", name="po")
                for m in range(2):
                    nc.tensor.matmul(p[:], aoT[m][:, tt * 128:(tt + 1) * 128],
                                     wo_s[m][:, nn * 512:(nn + 1) * 512],
                                     start=(m == 0), stop=(m == 1))
                ev = misc.tile([128, 512], F32, tag="oev", name="ev")
                nc.scalar.activation(ev[:], p[:],
                                     mybir.ActivationFunctionType.Copy)
                nc.sync.dma_start(outp_d[tt, nn], ev[:])

    nc.compile()
    return nc


def _mask_schedule(mask):
    """Classify (k-block 128) x (q-chunk 512) tiles from mask (S,S), 1=masked."""
    masked = mask >= 0.5
    schedule = []
    tiles = []
    tile_index = {}
    for c in range(NCHK):
        blocks = []
        for kb in range(NTOK):
            sub = masked[c * 512:(c + 1) * 512, kb * 128:(kb + 1) * 128]  # (q, k)
            if sub.all():
                continue
            if not sub.any():
                blocks.append((kb, None))
                continue
            t = np.ascontiguousarray((~sub).T.astype(np.float32))  # (k128, q512)
            key = t.tobytes()
            if key not in tile_index:
                tile_index[key] = len(tiles)
                tiles.append(t)
            blocks.append((kb, tile_index[key]))
        schedule.append(blocks)
    return schedule, tiles


def _block_T(x):
    """(S, DM) -> transposed blocked [KT, NHC, 128, 256] contiguous."""
    # xT[d, s]: block [kt, hc, p, j] = x[hc*256+j, kt*128+p]
    return np.ascontiguousarray(
        x.T.reshape(KT, 128, NHC, 256).transpose(0, 2, 1, 3))


def kernel(q, k, v, mask, Wq, bq, Wk, bk, Wv, bv, Wo, bo, _trace=False):
    q = np.asarray(q, np.float32)
    k = np.asarray(k, np.float32)
    v = np.asarray(v, np.float32)
    mask2d = np.asarray(mask, np.float32).reshape(S, S)
    Wq, Wk, Wv, Wo = (np.asarray(x, np.float32) for x in (Wq, Wk, Wv, Wo))
    bq, bk, bv, bo = (np.asarray(x, np.float32) for x in (bq, bk, bv, bo))

    use_f32r = USE_F32R
    ndt = np.float32 if use_f32r else ml_dtypes.bfloat16

    schedule, mtiles = _mask_schedule(mask2d)
    key = (tuple(tuple(b) for b in schedule), len(mtiles), use_f32r)
    if key not in _CACHE:
        _CACHE[key] = _build_program(schedule, len(mtiles), use_f32r)
    nc = _CACHE[key]

    mt_arr = np.stack(mtiles).astype(ndt) if mtiles else None
    in_maps = []
    for c in range(8):
        b, g = c // 4, c % 4
        hs = slice(g * HD, (g + 1) * HD)
        im = {
            "qT": _block_T(q[b]).astype(ndt),
            "kT": _block_T(k[b]).astype(ndt),
            "vT": _block_T(v[b]).astype(ndt),
            "wq": np.ascontiguousarray(Wq[:, hs]).astype(ndt),
            "wk": np.ascontiguousarray(Wk[:, hs]).astype(ndt),
            "wv": np.ascontiguousarray(Wv[:, hs]).astype(ndt),
            "wo": np.ascontiguousarray(Wo[hs, :]).astype(ndt),
            "bq": np.ascontiguousarray(bq[hs].reshape(HD, 1)),
            "bk": np.ascontiguousarray(bk[hs].reshape(HD, 1)),
            "bv": np.ascontiguousarray(bv[hs].reshape(1, HD)),
        }
        if mt_arr is not None:
            im["mt"] = mt_arr
        in_maps.append(im)

    res = bass_utils.run_bass_kernel_spmd(nc, in_maps, core_ids=list(range(8)),
                                          trace=_trace)

    out = np.zeros((B, S, DM), np.float32)
    attn = np.empty((B, NHEADS, S, S), np.float32)
    for c in range(8):
        b, g = c // 4, c % 4
        r = res.results[c]
        op = np.asarray(r["outp"], np.float32)  # [NTOK, 2, 128, 512]
        out[b] += op.transpose(0, 2, 1, 3).reshape(S, DM)
        at = np.asarray(r["attnT"]).astype(np.float32)  # [NH, NTOK, NCHK, 128, 512]
        sums = np.asarray(r["sums"], np.float32).reshape(NH, S)  # per-q row sums
        for h in range(NH):
            # blocked [kb, c, kl, qj] -> attn[q, k]: q=c*512+qj, k=kb*128+kl
            a = at[h].transpose(1, 3, 0, 2).reshape(S, S)
            a /= sums[h][:, None]
            attn[b, g * NH + h] = a
    out += bo
    if _trace:
        kernel.last_results = res
    return out, attn


# revision 11
# speedup vs baseline: 2.0844x; 1.4760x over previous
"""Multi-head attention (B=2, S=2048, D=1024, H=16, causal) on 8 TRN2 NeuronCores.

Sharding: 8 shards = 2 batches x 4 head-groups (4 heads each). Each core:
  - projects q/k/v for its batch through its head-group's weight slices
    (qhT/khT computed transposed: [head_dim, tok]; vh natural: [tok, head_dim])
  - computes causal attention per head in the transposed layout
    ST[k_tok, q_tok] = Kh @ Qh^T, exp (no max-subtraction: logits are O(1) and
    masked entries are never computed), a column of ones appended to the AV
    stationary yields the softmax denominators for free,
  - normalizes, writes attn^T per head (only causal blocks; the rest of the
    output buffer stays zero), in a DMA-contiguous blocked layout,
  - computes its partial output projection out_part = (attn @ vh) @ Wo_slice.
Host: shards/transposes inputs into blocked layouts, sums the 4 partial
outputs per batch, adds bo, and un-blocks/transposes attn back.

Matmul dtype: bf16 (full-rate PE) by default; float32r (half-rate, ~1.6e-4)
via KERNEL_F32R=1. PSUM accumulation is fp32 in both.
"""
import os
import sys

sys.path.insert(0, "/opt/trn_rl_repo")

import numpy as np
import ml_dtypes
from contextlib import ExitStack

import concourse.bass as bass
import concourse.tile as tile
from concourse import bacc, mybir
from concourse import bass_utils

F32 = mybir.dt.float32
F32R = mybir.dt.float32r
BF16 = mybir.dt.bfloat16

B, S, DM, NHEADS = 2, 2048, 1024, 16
NH = 4             # heads per core
DH = DM // NHEADS  # 64
HD = NH * DH       # 256 head dims per core
KT = DM // 128     # 8 contraction tiles for projections
NCHK = S // 512    # 4 q-chunks
NHC = S // 256     # 8 projection half-chunks
NTOK = S // 128    # 16 token tiles / k-blocks
SCALE = 1.0 / np.sqrt(np.float32(DH))

USE_F32R = os.environ.get("KERNEL_F32R", "0") == "1"

_CACHE = {}


def _build_program(schedule, nmask, use_f32r):
    """schedule: per chunk c, list of (kb, mask_idx|None). Same on all cores."""
    DT = F32R if use_f32r else BF16
    nc = bacc.Bacc("TRN2", target_bir_lowering=False, debug=False)

    # blocked inputs: [KT, NHC, 128, 256] so each DMA'd tile is contiguous
    qT_d = nc.dram_tensor("qT", [NHC, 128, KT, 256], DT, kind="ExternalInput").ap()
    kT_d = nc.dram_tensor("kT", [NHC, 128, KT, 256], DT, kind="ExternalInput").ap()
    vT_d = nc.dram_tensor("vT", [NHC, 128, KT, 256], DT, kind="ExternalInput").ap()
    wq_d = nc.dram_tensor("wq", [128, KT, HD], DT, kind="ExternalInput").ap()
    wk_d = nc.dram_tensor("wk", [128, KT, HD], DT, kind="ExternalInput").ap()
    wv_d = nc.dram_tensor("wv", [128, KT, HD], DT, kind="ExternalInput").ap()
    wo_d = nc.dram_tensor("wo", [HD, DM], DT, kind="ExternalInput").ap()
    bq_d = nc.dram_tensor("bq", [HD, 1], F32, kind="ExternalInput").ap()
    bk_d = nc.dram_tensor("bk", [HD, 1], F32, kind="ExternalInput").ap()
    bv_d = nc.dram_tensor("bv", [1, HD], F32, kind="ExternalInput").ap()
    mt_d = None
    if nmask:
        mt_d = nc.dram_tensor("mt", [nmask, 128, 512], DT, kind="ExternalInput").ap()

    # blocked outputs: every [128, 512] store is one contiguous region
    attnT_d = nc.dram_tensor("attnT", [NH, NCHK, 128, NTOK, 512], DT,
                             kind="ExternalOutput").ap()
    outp_d = nc.dram_tensor("outp", [NTOK, 128, 2, 512], F32,
                            kind="ExternalOutput").ap()
    sums_d = nc.dram_tensor("sums", [NH, NCHK, 1, 512], F32,
                            kind="ExternalOutput").ap()

    with tile.TileContext(nc) as tc, ExitStack() as ctx:
        wpool = ctx.enter_context(tc.tile_pool(name="w", bufs=1))
        stream = ctx.enter_context(tc.tile_pool(name="stream", bufs=2))
        stexp = ctx.enter_context(tc.tile_pool(name="stexp", bufs=2))
        misc = ctx.enter_context(tc.tile_pool(name="misc", bufs=2))
        psA = ctx.enter_context(tc.tile_pool(name="psA", bufs=2, space="PSUM"))
        psS = ctx.enter_context(tc.tile_pool(name="psS", bufs=3, space="PSUM"))
        psO = ctx.enter_context(tc.tile_pool(name="psO", bufs=2, space="PSUM"))

        # ---- persistent constants ------------------------------------------------
        wqkv = {}
        for name, srcd in (("wq", wq_d), ("wk", wk_d), ("wv", wv_d)):
            t = wpool.tile([128, KT, HD], DT, tag=name, name=name)
            nc.sync.dma_start(t[:], srcd)
            wqkv[name] = t
        wq_s = [wqkv["wq"][:, kt, :] for kt in range(KT)]
        wk_s = [wqkv["wk"][:, kt, :] for kt in range(KT)]
        wv_s = [wqkv["wv"][:, kt, :] for kt in range(KT)]
        wo_s = []
        for m in range(2):
            t = wpool.tile([128, DM], DT, tag=f"wo{m}", name=f"wo{m}")
            nc.sync.dma_start(t[:], wo_d[m * 128:(m + 1) * 128, :])
            wo_s.append(t)
        bqt, bkt = [], []
        for m in range(2):
            t = wpool.tile([128, 1], F32, tag=f"bq{m}", name=f"bq{m}")
            nc.sync.dma_start(t[:], bq_d[m * 128:(m + 1) * 128, :])
            bqt.append(t)
            t = wpool.tile([128, 1], F32, tag=f"bk{m}", name=f"bk{m}")
            nc.sync.dma_start(t[:], bk_d[m * 128:(m + 1) * 128, :])
            bkt.append(t)
        bv_row = wpool.tile([1, HD], F32, tag="bvrow")
        nc.sync.dma_start(bv_row[:], bv_d)
        bv_bc = wpool.tile([128, HD], F32, tag="bvbc")
        nc.gpsimd.partition_broadcast(bv_bc[:], bv_row[:])

        mt_s = []
        for i in range(nmask):
            t = wpool.tile([128, 512], DT, tag=f"mt{i}", name=f"mt{i}")
            nc.sync.dma_start(t[:], mt_d[i])
            mt_s.append(t)

        ones_f32 = wpool.tile([128, 1], F32, tag="ones")
        nc.vector.memset(ones_f32[:], 1.0)

        qhT = [wpool.tile([128, S], DT, tag=f"qhT{m}", name=f"qhT{m}")
               for m in range(2)]
        khT = [wpool.tile([128, S], DT, tag=f"khT{m}", name=f"khT{m}")
               for m in range(2)]
        aoT = [wpool.tile([128, S], DT, tag=f"aoT{m}", name=f"aoT{m}")
               for m in range(2)]
        vh_pack = [wpool.tile([128, NTOK * (DH + 1)], DT, tag=f"vhp{h}",
                              name=f"vhp{h}") for h in range(NH)]

        def vh_ones(h, tt):
            return vh_pack[h][:, tt * (DH + 1):(tt + 1) * (DH + 1)]

        for h in range(NH):
            for tt in range(NTOK):
                nc.vector.tensor_copy(vh_ones(h, tt)[:, DH:DH + 1], ones_f32[:])

        # ---- phase 2a: q/k projections (transposed: [head_dim, tok]) ------------
        for hc in range(NHC):
            cs = slice(hc * 256, (hc + 1) * 256)
            qin = stream.tile([128, KT, 256], DT, tag="qin", name="qin")
            nc.sync.dma_start(qin[:], qT_d[hc])
            kin = stream.tile([128, KT, 256], DT, tag="kin", name="kin")
            nc.sync.dma_start(kin[:], kT_d[hc])
            qt = [qin[:, kt, :] for kt in range(KT)]
            ktt = [kin[:, kt, :] for kt in range(KT)]
            for m in range(2):
                p = psA.tile([128, 256], F32, tag="psA", name="pq")
                for kt in range(KT):
                    nc.tensor.matmul(p[:], wq_s[kt][:, m * 128:(m + 1) * 128],
                                     qt[kt][:], start=(kt == 0), stop=(kt == KT - 1))
                nc.scalar.activation(qhT[m][:, cs], p[:],
                                     mybir.ActivationFunctionType.Identity,
                                     bias=bqt[m][:])
                p = psA.tile([128, 256], F32, tag="psA", name="pk")
                for kt in range(KT):
                    nc.tensor.matmul(p[:], wk_s[kt][:, m * 128:(m + 1) * 128],
                                     ktt[kt][:], start=(kt == 0), stop=(kt == KT - 1))
                nc.scalar.activation(khT[m][:, cs], p[:],
                                     mybir.ActivationFunctionType.Identity,
                                     bias=bkt[m][:])

        # ---- phase 2b: v projection (natural: [tok, head_dim]) ------------------
        for hc in range(NHC):
            vin = stream.tile([128, KT, 256], DT, tag="qin", name="vin")
            nc.sync.dma_start(vin[:], vT_d[hc])
            vt = [vin[:, kt, :] for kt in range(KT)]
            for tl in range(2):
                tt = hc * 2 + tl
                p = psA.tile([128, HD], F32, tag="psA", name="pv")
                for kt in range(KT):
                    nc.tensor.matmul(p[:], vt[kt][:, tl * 128:(tl + 1) * 128],
                                     wv_s[kt][:], start=(kt == 0), stop=(kt == KT - 1))
                pb = misc.tile([128, HD], F32, tag="vbias", name="pb")
                nc.vector.tensor_add(pb[:], p[:], bv_bc[:])
                for h in range(NH):
                    nc.vector.tensor_copy(vh_ones(h, tt)[:, 0:DH],
                                          pb[:, h * DH:(h + 1) * DH])

        # ---- phase 3: attention --------------------------------------------------
        for h in range(NH):
            m, po = h // 2, (h % 2) * 64
            for c in range(NCHK):
                blocks = schedule[c]
                cs = slice(c * 512, (c + 1) * 512)
                pOut = psO.tile([DH + 1, 512], F32, tag="psO", name="pOut")
                nkb = len(blocks)
                stc = stexp.tile([128, NTOK, 512], DT, tag="st", name="stc")
                for i, (kb, mi) in enumerate(blocks):
                    pS = psS.tile([128, 512], F32, tag="psS", name="pS")
                    nc.tensor.matmul(pS[:],
                                     khT[m][po:po + 64, kb * 128:(kb + 1) * 128],
                                     qhT[m][po:po + 64, cs],
                                     start=True, stop=True)
                    st = stc[:, i, :]
                    nc.scalar.activation(st[:], pS[:],
                                         mybir.ActivationFunctionType.Exp,
                                         scale=float(SCALE))
                    if mi is not None:
                        nc.vector.tensor_mul(st[:], st[:], mt_s[mi][:])
                    nc.tensor.matmul(pOut[:], vh_ones(h, kb)[:], st[:],
                                     start=(i == 0), stop=(i == len(blocks) - 1))
                # grouped stores: blocks are kb=0..nkb-1 in order; split across queues
                ngrp = min(4, nkb)
                bnds = [round(j * nkb / ngrp) for j in range(ngrp + 1)]
                for j in range(ngrp):
                    lo, hi = bnds[j], bnds[j + 1]
                    if hi > lo:
                        nc.sync.dma_start(attnT_d[h, c, :, lo:hi, :],
                                          stc[:, lo:hi, :])
                sumrow = misc.tile([128, 512], F32, tag="sumrow", name="sumrow")
                nc.scalar.activation(sumrow[64:65, :], pOut[DH:DH + 1, :],
                                     mybir.ActivationFunctionType.Copy)
                nc.sync.dma_start(sums_d[h, c], sumrow[64:65, :])
                row0 = misc.tile([1, 512], F32, tag="row0", name="row0")
                nc.sync.dma_start(row0[:], sumrow[64:65, :])
                recip0 = misc.tile([1, 512], F32, tag="recip0", name="recip0")
                nc.vector.reciprocal_approx_fast(recip0[:], row0[:])
                recip_bc = misc.tile([64, 512], F32, tag="recipbc", name="recipbc")
                nc.gpsimd.partition_broadcast(recip_bc[:], recip0[:])
                nc.vector.tensor_mul(aoT[m][po:po + 64, cs], pOut[0:DH, :],
                                     recip_bc[:])

        # ---- phase 4: output projection -----------------------------------------
        for tt in range(NTOK):
            ev = misc.tile([128, 2, 512], F32, tag="oev", name="ev")
            for nn in range(2):
                p = psA.tile([128, 512], F32, tag="psA", name="po")
                for m in range(2):
                    nc.tensor.matmul(p[:], aoT[m][:, tt * 128:(tt + 1) * 128],
                                     wo_s[m][:, nn * 512:(nn + 1) * 512],
                                     start=(m == 0), stop=(m == 1))
                nc.vector.tensor_copy(ev[:, nn, :], p[:])
            nc.sync.dma_start(outp_d[tt], ev[:])

    nc.compile()
    return nc


def _mask_schedule(mask):
    """Classify (k-block 128) x (q-chunk 512) tiles from mask (S,S), 1=masked."""
    masked = mask >= 0.5
    schedule = []
    tiles = []
    tile_index = {}
    for c in range(NCHK):
        blocks = []
        for kb in range(NTOK):
            sub = masked[c * 512:(c + 1) * 512, kb * 128:(kb + 1) * 128]  # (q, k)
            if sub.all():
                continue
            if not sub.any():
                blocks.append((kb, None))
                continue
            t = np.ascontiguousarray((~sub).T.astype(np.float32))  # (k128, q512)
            key = t.tobytes()
            if key not in tile_index:
                tile_index[key] = len(tiles)
                tiles.append(t)
            blocks.append((kb, tile_index[key]))
        schedule.append(blocks)
    return schedule, tiles


def _block_T(x):
    """(S, DM) -> transposed blocked [NHC, 128, KT, 256] contiguous."""
    # block [hc, p, kt, j] = x[hc*256+j, kt*128+p]
    return np.ascontiguousarray(
        x.T.reshape(KT, 128, NHC, 256).transpose(2, 1, 0, 3))


def kernel(q, k, v, mask, Wq, bq, Wk, bk, Wv, bv, Wo, bo, _trace=False):
    q = np.asarray(q, np.float32)
    k = np.asarray(k, np.float32)
    v = np.asarray(v, np.float32)
    mask2d = np.asarray(mask, np.float32).reshape(S, S)
    Wq, Wk, Wv, Wo = (np.asarray(x, np.float32) for x in (Wq, Wk, Wv, Wo))
    bq, bk, bv, bo = (np.asarray(x, np.float32) for x in (bq, bk, bv, bo))

    use_f32r = USE_F32R
    ndt = np.float32 if use_f32r else ml_dtypes.bfloat16

    schedule, mtiles = _mask_schedule(mask2d)
    key = (tuple(tuple(b) for b in schedule), len(mtiles), use_f32r)
    if key not in _CACHE:
        _CACHE[key] = _build_program(schedule, len(mtiles), use_f32r)
    nc = _CACHE[key]

    mt_arr = np.stack(mtiles).astype(ndt) if mtiles else None
    in_maps = []
    for c in range(8):
        b, g = c // 4, c % 4
        hs = slice(g * HD, (g + 1) * HD)
        im = {
            "qT": _block_T(q[b]).astype(ndt),
            "kT": _block_T(k[b]).astype(ndt),
            "vT": _block_T(v[b]).astype(ndt),
            "wq": np.ascontiguousarray(
                Wq[:, hs].reshape(KT, 128, HD).transpose(1, 0, 2)).astype(ndt),
            "wk": np.ascontiguousarray(
                Wk[:, hs].reshape(KT, 128, HD).transpose(1, 0, 2)).astype(ndt),
            "wv": np.ascontiguousarray(
                Wv[:, hs].reshape(KT, 128, HD).transpose(1, 0, 2)).astype(ndt),
            "wo": np.ascontiguousarray(Wo[hs, :]).astype(ndt),
            "bq": np.ascontiguousarray(bq[hs].reshape(HD, 1)),
            "bk": np.ascontiguousarray(bk[hs].reshape(HD, 1)),
            "bv": np.ascontiguousarray(bv[hs].reshape(1, HD)),
        }
        if mt_arr is not None:
            im["mt"] = mt_arr
        in_maps.append(im)

    res = bass_utils.run_bass_kernel_spmd(nc, in_maps, core_ids=list(range(8)),
                                          trace=_trace)

    out = np.zeros((B, S, DM), np.float32)
    attn = np.empty((B, NHEADS, S, S), np.float32)
    for c in range(8):
        b, g = c // 4, c % 4
        r = res.results[c]
        op = np.asarray(r["outp"], np.float32)  # [NTOK, 128, 2, 512]
        out[b] += op.reshape(S, DM)
        at = np.asarray(r["attnT"]).astype(np.float32)  # [NH, NCHK, 128, NTOK, 512]
        sums = np.asarray(r["sums"], np.float32).reshape(NH, S)  # per-q row sums
        for h in range(NH):
            # slot i in the NTOK axis holds k-block schedule[c][i][0]
            a = np.zeros((S, S), np.float32)
            for ci in range(NCHK):
                blk = at[h, ci].transpose(1, 0, 2)  # [NTOK, 128(kl), 512(qj)]
                for i, (kb, _mi) in enumerate(schedule[ci]):
                    a[ci * 512:(ci + 1) * 512, kb * 128:(kb + 1) * 128] = blk[i].T
            a /= sums[h][:, None]
            attn[b, g * NH + h] = a
    out += bo
    if _trace:
        kernel.last_results = res
    return out, attn


# revision 12
# speedup vs baseline: 2.1795x; 1.0456x over previous
"""Multi-head attention (B=2, S=2048, D=1024, H=16, causal) on 8 TRN2 NeuronCores.

Sharding: 8 shards = 2 batches x 4 head-groups (4 heads each). Each core:
  - projects q/k/v for its batch through its head-group's weight slices
    (qhT/khT computed transposed: [head_dim, tok]; vh natural: [tok, head_dim])
  - computes causal attention per head in the transposed layout
    ST[k_tok, q_tok] = Kh @ Qh^T, exp (no max-subtraction: logits are O(1) and
    masked entries are never computed), a column of ones appended to the AV
    stationary yields the softmax denominators for free,
  - normalizes, writes attn^T per head (only causal blocks; the rest of the
    output buffer stays zero), in a DMA-contiguous blocked layout,
  - computes its partial output projection out_part = (attn @ vh) @ Wo_slice.
Host: shards/transposes inputs into blocked layouts, sums the 4 partial
outputs per batch, adds bo, and un-blocks/transposes attn back.

Matmul dtype: bf16 (full-rate PE) by default; float32r (half-rate, ~1.6e-4)
via KERNEL_F32R=1. PSUM accumulation is fp32 in both.
"""
import os
import sys

sys.path.insert(0, "/opt/trn_rl_repo")

import numpy as np
import ml_dtypes
from contextlib import ExitStack

import concourse.bass as bass
import concourse.tile as tile
from concourse import bacc, mybir
from concourse import bass_utils

F32 = mybir.dt.float32
F32R = mybir.dt.float32r
BF16 = mybir.dt.bfloat16

B, S, DM, NHEADS = 2, 2048, 1024, 16
NH = 4             # heads per core
DH = DM // NHEADS  # 64
HD = NH * DH       # 256 head dims per core
KT = DM // 128     # 8 contraction tiles for projections
NCHK = S // 512    # 4 q-chunks
NHC = S // 256     # 8 projection half-chunks
NTOK = S // 128    # 16 token tiles / k-blocks
SCALE = 1.0 / np.sqrt(np.float32(DH))

USE_F32R = os.environ.get("KERNEL_F32R", "0") == "1"

_CACHE = {}


def _build_program(schedule, nmask, use_f32r):
    """schedule: per chunk c, list of (kb, mask_idx|None). Same on all cores."""
    DT = F32R if use_f32r else BF16
    nc = bacc.Bacc("TRN2", target_bir_lowering=False, debug=False)

    # blocked inputs: [KT, NHC, 128, 256] so each DMA'd tile is contiguous
    qT_d = nc.dram_tensor("qT", [NCHK, 128, KT, 512], DT, kind="ExternalInput").ap()
    kT_d = nc.dram_tensor("kT", [NCHK, 128, KT, 512], DT, kind="ExternalInput").ap()
    vT_d = nc.dram_tensor("vT", [NCHK, 128, KT, 512], DT, kind="ExternalInput").ap()
    wq_d = nc.dram_tensor("wq", [128, KT, HD], DT, kind="ExternalInput").ap()
    wk_d = nc.dram_tensor("wk", [128, KT, HD], DT, kind="ExternalInput").ap()
    wv_d = nc.dram_tensor("wv", [128, KT, HD], DT, kind="ExternalInput").ap()
    wo_d = nc.dram_tensor("wo", [HD, DM], DT, kind="ExternalInput").ap()
    bq_d = nc.dram_tensor("bq", [HD, 1], F32, kind="ExternalInput").ap()
    bk_d = nc.dram_tensor("bk", [HD, 1], F32, kind="ExternalInput").ap()
    bv_d = nc.dram_tensor("bv", [1, HD], F32, kind="ExternalInput").ap()
    mt_d = None
    if nmask:
        mt_d = nc.dram_tensor("mt", [nmask, 128, 512], DT, kind="ExternalInput").ap()

    # blocked outputs: every [128, 512] store is one contiguous region
    attnT_d = nc.dram_tensor("attnT", [NH, NCHK, 128, NTOK, 512], DT,
                             kind="ExternalOutput").ap()
    outp_d = nc.dram_tensor("outp", [NTOK, 128, 2, 512], F32,
                            kind="ExternalOutput").ap()
    sums_d = nc.dram_tensor("sums", [NH, NCHK, 1, 512], F32,
                            kind="ExternalOutput").ap()

    with tile.TileContext(nc) as tc, ExitStack() as ctx:
        wpool = ctx.enter_context(tc.tile_pool(name="w", bufs=1))
        stream = ctx.enter_context(tc.tile_pool(name="stream", bufs=2))
        stexp = ctx.enter_context(tc.tile_pool(name="stexp", bufs=3))
        misc = ctx.enter_context(tc.tile_pool(name="misc", bufs=2))
        psA = ctx.enter_context(tc.tile_pool(name="psA", bufs=2, space="PSUM"))
        psS = ctx.enter_context(tc.tile_pool(name="psS", bufs=4, space="PSUM"))
        psO = ctx.enter_context(tc.tile_pool(name="psO", bufs=2, space="PSUM"))

        # ---- persistent constants ------------------------------------------------
        wqkv = {}
        for name, srcd in (("wq", wq_d), ("wk", wk_d), ("wv", wv_d)):
            t = wpool.tile([128, KT, HD], DT, tag=name, name=name)
            nc.sync.dma_start(t[:], srcd)
            wqkv[name] = t
        wq_s = [wqkv["wq"][:, kt, :] for kt in range(KT)]
        wk_s = [wqkv["wk"][:, kt, :] for kt in range(KT)]
        wv_s = [wqkv["wv"][:, kt, :] for kt in range(KT)]
        wo_s = []
        for m in range(2):
            t = wpool.tile([128, DM], DT, tag=f"wo{m}", name=f"wo{m}")
            nc.sync.dma_start(t[:], wo_d[m * 128:(m + 1) * 128, :])
            wo_s.append(t)
        bqt, bkt = [], []
        for m in range(2):
            t = wpool.tile([128, 1], F32, tag=f"bq{m}", name=f"bq{m}")
            nc.sync.dma_start(t[:], bq_d[m * 128:(m + 1) * 128, :])
            bqt.append(t)
            t = wpool.tile([128, 1], F32, tag=f"bk{m}", name=f"bk{m}")
            nc.sync.dma_start(t[:], bk_d[m * 128:(m + 1) * 128, :])
            bkt.append(t)
        bv_row = wpool.tile([1, HD], F32, tag="bvrow")
        nc.sync.dma_start(bv_row[:], bv_d)
        bv_bc = wpool.tile([128, HD], F32, tag="bvbc")
        nc.gpsimd.partition_broadcast(bv_bc[:], bv_row[:])

        mt_s = []
        for i in range(nmask):
            t = wpool.tile([128, 512], DT, tag=f"mt{i}", name=f"mt{i}")
            nc.sync.dma_start(t[:], mt_d[i])
            mt_s.append(t)

        ones_f32 = wpool.tile([128, 1], F32, tag="ones")
        nc.vector.memset(ones_f32[:], 1.0)

        qhT = [wpool.tile([128, S], DT, tag=f"qhT{m}", name=f"qhT{m}")
               for m in range(2)]
        khT = [wpool.tile([128, S], DT, tag=f"khT{m}", name=f"khT{m}")
               for m in range(2)]
        aoT = [wpool.tile([128, S], DT, tag=f"aoT{m}", name=f"aoT{m}")
               for m in range(2)]
        vh_pack = [wpool.tile([128, NTOK * (DH + 1)], DT, tag=f"vhp{h}",
                              name=f"vhp{h}") for h in range(NH)]

        def vh_ones(h, tt):
            return vh_pack[h][:, tt * (DH + 1):(tt + 1) * (DH + 1)]

        for h in range(NH):
            for tt in range(NTOK):
                nc.vector.tensor_copy(vh_ones(h, tt)[:, DH:DH + 1], ones_f32[:])

        # ---- phase 2a: q/k projections (transposed: [head_dim, tok]) ------------
        for cc in range(NCHK):
            cs = slice(cc * 512, (cc + 1) * 512)
            qin = stream.tile([128, KT, 512], DT, tag="qin", name="qin")
            nc.sync.dma_start(qin[:], qT_d[cc])
            kin = stream.tile([128, KT, 512], DT, tag="kin", name="kin")
            nc.sync.dma_start(kin[:], kT_d[cc])
            for m in range(2):
                p = psA.tile([128, 512], F32, tag="psA", name="pq")
                for kt in range(KT):
                    nc.tensor.matmul(p[:], wq_s[kt][:, m * 128:(m + 1) * 128],
                                     qin[:, kt, :], start=(kt == 0),
                                     stop=(kt == KT - 1))
                nc.scalar.activation(qhT[m][:, cs], p[:],
                                     mybir.ActivationFunctionType.Identity,
                                     bias=bqt[m][:])
                p = psA.tile([128, 512], F32, tag="psA", name="pk")
                for kt in range(KT):
                    nc.tensor.matmul(p[:], wk_s[kt][:, m * 128:(m + 1) * 128],
                                     kin[:, kt, :], start=(kt == 0),
                                     stop=(kt == KT - 1))
                nc.scalar.activation(khT[m][:, cs], p[:],
                                     mybir.ActivationFunctionType.Identity,
                                     bias=bkt[m][:])

        # ---- phase 2b: v projection (natural: [tok, head_dim]) ------------------
        for cc in range(NCHK):
            vin = stream.tile([128, KT, 512], DT, tag="qin", name="vin")
            nc.sync.dma_start(vin[:], vT_d[cc])
            for tl in range(4):
                tt = cc * 4 + tl
                p = psA.tile([128, HD], F32, tag="psA", name="pv")
                for kt in range(KT):
                    nc.tensor.matmul(p[:], vin[:, kt, tl * 128:(tl + 1) * 128],
                                     wv_s[kt][:], start=(kt == 0),
                                     stop=(kt == KT - 1))
                pb = misc.tile([128, HD], F32, tag="vbias", name="pb")
                nc.vector.tensor_add(pb[:], p[:], bv_bc[:])
                for h in range(NH):
                    nc.vector.tensor_copy(vh_ones(h, tt)[:, 0:DH],
                                          pb[:, h * DH:(h + 1) * DH])

        # ---- phase 3: attention --------------------------------------------------
        for h in range(NH):
            m, po = h // 2, (h % 2) * 64
            for c in range(NCHK):
                blocks = schedule[c]
                cs = slice(c * 512, (c + 1) * 512)
                pOut = psO.tile([DH + 1, 512], F32, tag="psO", name="pOut")
                nkb = len(blocks)
                stc = stexp.tile([128, NTOK, 512], DT, tag="st", name="stc")
                for i, (kb, mi) in enumerate(blocks):
                    pS = psS.tile([128, 512], F32, tag="psS", name="pS")
                    nc.tensor.matmul(pS[:],
                                     khT[m][po:po + 64, kb * 128:(kb + 1) * 128],
                                     qhT[m][po:po + 64, cs],
                                     start=True, stop=True)
                    st = stc[:, i, :]
                    nc.scalar.activation(st[:], pS[:],
                                         mybir.ActivationFunctionType.Exp,
                                         scale=float(SCALE))
                    if mi is not None:
                        nc.vector.tensor_mul(st[:], st[:], mt_s[mi][:])
                    nc.tensor.matmul(pOut[:], vh_ones(h, kb)[:], st[:],
                                     start=(i == 0), stop=(i == len(blocks) - 1))
                # grouped stores: blocks are kb=0..nkb-1 in order; split across queues
                ngrp = min(4, nkb)
                bnds = [round(j * nkb / ngrp) for j in range(ngrp + 1)]
                for j in range(ngrp):
                    lo, hi = bnds[j], bnds[j + 1]
                    if hi > lo:
                        nc.sync.dma_start(attnT_d[h, c, :, lo:hi, :],
                                          stc[:, lo:hi, :])
                sumrow = misc.tile([128, 512], F32, tag="sumrow", name="sumrow")
                nc.scalar.activation(sumrow[64:65, :], pOut[DH:DH + 1, :],
                                     mybir.ActivationFunctionType.Copy)
                nc.sync.dma_start(sums_d[h, c], sumrow[64:65, :])
                row0 = misc.tile([1, 512], F32, tag="row0", name="row0")
                nc.sync.dma_start(row0[:], sumrow[64:65, :])
                recip0 = misc.tile([1, 512], F32, tag="recip0", name="recip0")
                nc.vector.reciprocal_approx_fast(recip0[:], row0[:])
                recip_bc = misc.tile([64, 512], F32, tag="recipbc", name="recipbc")
                nc.gpsimd.partition_broadcast(recip_bc[:], recip0[:])
                nc.vector.tensor_mul(aoT[m][po:po + 64, cs], pOut[0:DH, :],
                                     recip_bc[:])

        # ---- phase 4: output projection -----------------------------------------
        for tt in range(NTOK):
            ev = misc.tile([128, 2, 512], F32, tag="oev", name="ev")
            for nn in range(2):
                p = psA.tile([128, 512], F32, tag="psA", name="po")
                for m in range(2):
                    nc.tensor.matmul(p[:], aoT[m][:, tt * 128:(tt + 1) * 128],
                                     wo_s[m][:, nn * 512:(nn + 1) * 512],
                                     start=(m == 0), stop=(m == 1))
                nc.vector.tensor_copy(ev[:, nn, :], p[:])
            nc.sync.dma_start(outp_d[tt], ev[:])

    nc.compile()
    return nc


def _mask_schedule(mask):
    """Classify (k-block 128) x (q-chunk 512) tiles from mask (S,S), 1=masked."""
    masked = mask >= 0.5
    schedule = []
    tiles = []
    tile_index = {}
    for c in range(NCHK):
        blocks = []
        for kb in range(NTOK):
            sub = masked[c * 512:(c + 1) * 512, kb * 128:(kb + 1) * 128]  # (q, k)
            if sub.all():
                continue
            if not sub.any():
                blocks.append((kb, None))
                continue
            t = np.ascontiguousarray((~sub).T.astype(np.float32))  # (k128, q512)
            key = t.tobytes()
            if key not in tile_index:
                tile_index[key] = len(tiles)
                tiles.append(t)
            blocks.append((kb, tile_index[key]))
        schedule.append(blocks)
    return schedule, tiles


def _block_T(x):
    """(S, DM) -> transposed blocked [NCHK, 128, KT, 512] contiguous."""
    # block [cc, p, kt, j] = x[cc*512+j, kt*128+p]
    return np.ascontiguousarray(
        x.T.reshape(KT, 128, NCHK, 512).transpose(2, 1, 0, 3))


def kernel(q, k, v, mask, Wq, bq, Wk, bk, Wv, bv, Wo, bo, _trace=False):
    q = np.asarray(q, np.float32)
    k = np.asarray(k, np.float32)
    v = np.asarray(v, np.float32)
    mask2d = np.asarray(mask, np.float32).reshape(S, S)
    Wq, Wk, Wv, Wo = (np.asarray(x, np.float32) for x in (Wq, Wk, Wv, Wo))
    bq, bk, bv, bo = (np.asarray(x, np.float32) for x in (bq, bk, bv, bo))

    use_f32r = USE_F32R
    ndt = np.float32 if use_f32r else ml_dtypes.bfloat16

    schedule, mtiles = _mask_schedule(mask2d)
    key = (tuple(tuple(b) for b in schedule), len(mtiles), use_f32r)
    if key not in _CACHE:
        _CACHE[key] = _build_program(schedule, len(mtiles), use_f32r)
    nc = _CACHE[key]

    mt_arr = np.stack(mtiles).astype(ndt) if mtiles else None
    in_maps = []
    for c in range(8):
        b, g = c // 4, c % 4
        hs = slice(g * HD, (g + 1) * HD)
        im = {
            "qT": _block_T(q[b]).astype(ndt),
            "kT": _block_T(k[b]).astype(ndt),
            "vT": _block_T(v[b]).astype(ndt),
            "wq": np.ascontiguousarray(
                Wq[:, hs].reshape(KT, 128, HD).transpose(1, 0, 2)).astype(ndt),
            "wk": np.ascontiguousarray(
                Wk[:, hs].reshape(KT, 128, HD).transpose(1, 0, 2)).astype(ndt),
            "wv": np.ascontiguousarray(
                Wv[:, hs].reshape(KT, 128, HD).transpose(1, 0, 2)).astype(ndt),
            "wo": np.ascontiguousarray(Wo[hs, :]).astype(ndt),
            "bq": np.ascontiguousarray(bq[hs].reshape(HD, 1)),
            "bk": np.ascontiguousarray(bk[hs].reshape(HD, 1)),
            "bv": np.ascontiguousarray(bv[hs].reshape(1, HD)),
        }
        if mt_arr is not None:
            im["mt"] = mt_arr
        in_maps.append(im)

    res = bass_utils.run_bass_kernel_spmd(nc, in_maps, core_ids=list(range(8)),
                                          trace=_trace)

    out = np.zeros((B, S, DM), np.float32)
    attn = np.empty((B, NHEADS, S, S), np.float32)
    for c in range(8):
        b, g = c // 4, c % 4
        r = res.results[c]
        op = np.asarray(r["outp"], np.float32)  # [NTOK, 128, 2, 512]
        out[b] += op.reshape(S, DM)
        at = np.asarray(r["attnT"]).astype(np.float32)  # [NH, NCHK, 128, NTOK, 512]
        sums = np.asarray(r["sums"], np.float32).reshape(NH, S)  # per-q row sums
        for h in range(NH):
            # slot i in the NTOK axis holds k-block schedule[c][i][0]
            a = np.zeros((S, S), np.float32)
            for ci in range(NCHK):
                blk = at[h, ci].transpose(1, 0, 2)  # [NTOK, 128(kl), 512(qj)]
                for i, (kb, _mi) in enumerate(schedule[ci]):
                    a[ci * 512:(ci + 1) * 512, kb * 128:(kb + 1) * 128] = blk[i].T
            a /= sums[h][:, None]
            attn[b, g * NH + h] = a
    out += bo
    if _trace:
        kernel.last_results = res
    return out, attn


# revision 13
# speedup vs baseline: 2.1822x; 1.0012x over previous
"""Multi-head attention (B=2, S=2048, D=1024, H=16, causal) on 8 TRN2 NeuronCores.

Sharding: 8 shards = 2 batches x 4 head-groups (4 heads each). Each core:
  - projects q/k/v for its batch through its head-group's weight slices
    (qhT/khT computed transposed: [head_dim, tok]; vh natural: [tok, head_dim])
  - computes causal attention per head in the transposed layout
    ST[k_tok, q_tok] = Kh @ Qh^T, exp (no max-subtraction: logits are O(1) and
    masked entries are never computed), a column of ones appended to the AV
    stationary yields the softmax denominators for free,
  - normalizes, writes attn^T per head (only causal blocks; the rest of the
    output buffer stays zero), in a DMA-contiguous blocked layout,
  - computes its partial output projection out_part = (attn @ vh) @ Wo_slice.
Host: shards/transposes inputs into blocked layouts, sums the 4 partial
outputs per batch, adds bo, and un-blocks/transposes attn back.

Matmul dtype: bf16 (full-rate PE) by default; float32r (half-rate, ~1.6e-4)
via KERNEL_F32R=1. PSUM accumulation is fp32 in both.
"""
import os
import sys

sys.path.insert(0, "/opt/trn_rl_repo")

import numpy as np
import ml_dtypes
from contextlib import ExitStack

import concourse.bass as bass
import concourse.tile as tile
from concourse import bacc, mybir
from concourse import bass_utils

F32 = mybir.dt.float32
F32R = mybir.dt.float32r
BF16 = mybir.dt.bfloat16

B, S, DM, NHEADS = 2, 2048, 1024, 16
NH = 4             # heads per core
DH = DM // NHEADS  # 64
HD = NH * DH       # 256 head dims per core
KT = DM // 128     # 8 contraction tiles for projections
NCHK = S // 512    # 4 q-chunks
NHC = S // 256     # 8 projection half-chunks
NTOK = S // 128    # 16 token tiles / k-blocks
SCALE = 1.0 / np.sqrt(np.float32(DH))

USE_F32R = os.environ.get("KERNEL_F32R", "0") == "1"

_CACHE = {}


def _build_program(schedule, nmask, use_f32r):
    """schedule: per chunk c, list of (kb, mask_idx|None). Same on all cores."""
    DT = F32R if use_f32r else BF16
    nc = bacc.Bacc("TRN2", target_bir_lowering=False, debug=False)

    # blocked inputs: [KT, NHC, 128, 256] so each DMA'd tile is contiguous
    qT_d = nc.dram_tensor("qT", [NCHK, 128, KT, 512], DT, kind="ExternalInput").ap()
    kT_d = nc.dram_tensor("kT", [NCHK, 128, KT, 512], DT, kind="ExternalInput").ap()
    vT_d = nc.dram_tensor("vT", [NCHK, 128, KT, 512], DT, kind="ExternalInput").ap()
    wq_d = nc.dram_tensor("wq", [128, KT, HD], DT, kind="ExternalInput").ap()
    wk_d = nc.dram_tensor("wk", [128, KT, HD], DT, kind="ExternalInput").ap()
    wv_d = nc.dram_tensor("wv", [128, KT, HD], DT, kind="ExternalInput").ap()
    wo_d = nc.dram_tensor("wo", [HD, DM], DT, kind="ExternalInput").ap()
    bq_d = nc.dram_tensor("bq", [HD, 1], F32, kind="ExternalInput").ap()
    bk_d = nc.dram_tensor("bk", [HD, 1], F32, kind="ExternalInput").ap()
    bv_d = nc.dram_tensor("bv", [1, HD], F32, kind="ExternalInput").ap()
    mt_d = None
    if nmask:
        mt_d = nc.dram_tensor("mt", [nmask, 128, 512], DT, kind="ExternalInput").ap()

    # blocked outputs: every [128, 512] store is one contiguous region
    attnT_d = nc.dram_tensor("attnT", [NH, NCHK, 128, NTOK, 512], DT,
                             kind="ExternalOutput").ap()
    outp_d = nc.dram_tensor("outp", [NTOK, 128, 2, 512], F32,
                            kind="ExternalOutput").ap()
    sums_d = nc.dram_tensor("sums", [NH, NCHK, 1, 512], F32,
                            kind="ExternalOutput").ap()

    with tile.TileContext(nc) as tc, ExitStack() as ctx:
        wpool = ctx.enter_context(tc.tile_pool(name="w", bufs=1))
        stream = ctx.enter_context(tc.tile_pool(name="stream", bufs=2 if not use_f32r else 1))
        stexp = ctx.enter_context(tc.tile_pool(name="stexp", bufs=3 if not use_f32r else 1))
        misc = ctx.enter_context(tc.tile_pool(name="misc", bufs=2))
        psA = ctx.enter_context(tc.tile_pool(name="psA", bufs=2, space="PSUM"))
        psS = ctx.enter_context(tc.tile_pool(name="psS", bufs=4, space="PSUM"))
        psO = ctx.enter_context(tc.tile_pool(name="psO", bufs=2, space="PSUM"))

        # ---- persistent constants ------------------------------------------------
        wqkv = {}
        for name, srcd in (("wq", wq_d), ("wk", wk_d), ("wv", wv_d)):
            t = wpool.tile([128, KT, HD], DT, tag=name, name=name)
            nc.sync.dma_start(t[:], srcd)
            wqkv[name] = t
        wq_s = [wqkv["wq"][:, kt, :] for kt in range(KT)]
        wk_s = [wqkv["wk"][:, kt, :] for kt in range(KT)]
        wv_s = [wqkv["wv"][:, kt, :] for kt in range(KT)]
        wo_s = []
        for m in range(2):
            t = wpool.tile([128, DM], DT, tag=f"wo{m}", name=f"wo{m}")
            nc.sync.dma_start(t[:], wo_d[m * 128:(m + 1) * 128, :])
            wo_s.append(t)
        bqt, bkt = [], []
        for m in range(2):
            t = wpool.tile([128, 1], F32, tag=f"bq{m}", name=f"bq{m}")
            nc.sync.dma_start(t[:], bq_d[m * 128:(m + 1) * 128, :])
            bqt.append(t)
            t = wpool.tile([128, 1], F32, tag=f"bk{m}", name=f"bk{m}")
            nc.sync.dma_start(t[:], bk_d[m * 128:(m + 1) * 128, :])
            bkt.append(t)
        bv_row = wpool.tile([1, HD], F32, tag="bvrow")
        nc.sync.dma_start(bv_row[:], bv_d)
        bv_bc = wpool.tile([128, HD], F32, tag="bvbc")
        nc.gpsimd.partition_broadcast(bv_bc[:], bv_row[:])

        mt_s = []
        for i in range(nmask):
            t = wpool.tile([128, 512], DT, tag=f"mt{i}", name=f"mt{i}")
            nc.sync.dma_start(t[:], mt_d[i])
            mt_s.append(t)

        ones_f32 = wpool.tile([128, 1], F32, tag="ones")
        nc.vector.memset(ones_f32[:], 1.0)

        qhT = [wpool.tile([128, S], DT, tag=f"qhT{m}", name=f"qhT{m}")
               for m in range(2)]
        khT = [wpool.tile([128, S], DT, tag=f"khT{m}", name=f"khT{m}")
               for m in range(2)]
        aoT = [wpool.tile([128, S], DT, tag=f"aoT{m}", name=f"aoT{m}")
               for m in range(2)]
        vh_pack = [wpool.tile([128, NTOK * (DH + 1)], DT, tag=f"vhp{h}",
                              name=f"vhp{h}") for h in range(NH)]

        def vh_ones(h, tt):
            return vh_pack[h][:, tt * (DH + 1):(tt + 1) * (DH + 1)]

        for h in range(NH):
            for tt in range(NTOK):
                nc.vector.tensor_copy(vh_ones(h, tt)[:, DH:DH + 1], ones_f32[:])

        # ---- phase 2a: q/k projections (transposed: [head_dim, tok]) ------------
        for cc in range(NCHK):
            cs = slice(cc * 512, (cc + 1) * 512)
            qin = stream.tile([128, KT, 512], DT, tag="qin", name="qin")
            nc.sync.dma_start(qin[:], qT_d[cc])
            kin = stream.tile([128, KT, 512], DT, tag="kin", name="kin")
            nc.sync.dma_start(kin[:], kT_d[cc])
            for m in range(2):
                p = psA.tile([128, 512], F32, tag="psA", name="pq")
                for kt in range(KT):
                    nc.tensor.matmul(p[:], wq_s[kt][:, m * 128:(m + 1) * 128],
                                     qin[:, kt, :], start=(kt == 0),
                                     stop=(kt == KT - 1))
                nc.scalar.activation(qhT[m][:, cs], p[:],
                                     mybir.ActivationFunctionType.Identity,
                                     bias=bqt[m][:])
                p = psA.tile([128, 512], F32, tag="psA", name="pk")
                for kt in range(KT):
                    nc.tensor.matmul(p[:], wk_s[kt][:, m * 128:(m + 1) * 128],
                                     kin[:, kt, :], start=(kt == 0),
                                     stop=(kt == KT - 1))
                nc.scalar.activation(khT[m][:, cs], p[:],
                                     mybir.ActivationFunctionType.Identity,
                                     bias=bkt[m][:])

        # ---- phase 2b: v projection (natural: [tok, head_dim]) ------------------
        for cc in range(NCHK):
            vin = stream.tile([128, KT, 512], DT, tag="qin", name="vin")
            nc.sync.dma_start(vin[:], vT_d[cc])
            for tl in range(4):
                tt = cc * 4 + tl
                p = psA.tile([128, HD], F32, tag="psA", name="pv")
                for kt in range(KT):
                    nc.tensor.matmul(p[:], vin[:, kt, tl * 128:(tl + 1) * 128],
                                     wv_s[kt][:], start=(kt == 0),
                                     stop=(kt == KT - 1))
                pb = misc.tile([128, HD], F32, tag="vbias", name="pb")
                nc.vector.tensor_add(pb[:], p[:], bv_bc[:])
                for h in range(NH):
                    nc.vector.tensor_copy(vh_ones(h, tt)[:, 0:DH],
                                          pb[:, h * DH:(h + 1) * DH])

        # ---- phase 3: attention --------------------------------------------------
        for h in range(NH):
            m, po = h // 2, (h % 2) * 64
            for c in range(NCHK):
                blocks = schedule[c]
                cs = slice(c * 512, (c + 1) * 512)
                pOut = psO.tile([DH + 1, 512], F32, tag="psO", name="pOut")
                nkb = len(blocks)
                stc = stexp.tile([128, NTOK, 512], DT, tag="st", name="stc")
                for i, (kb, mi) in enumerate(blocks):
                    pS = psS.tile([128, 512], F32, tag="psS", name="pS")
                    nc.tensor.matmul(pS[:],
                                     khT[m][po:po + 64, kb * 128:(kb + 1) * 128],
                                     qhT[m][po:po + 64, cs],
                                     start=True, stop=True)
                    st = stc[:, i, :]
                    nc.scalar.activation(st[:], pS[:],
                                         mybir.ActivationFunctionType.Exp,
                                         scale=float(SCALE))
                    if mi is not None:
                        nc.vector.tensor_mul(st[:], st[:], mt_s[mi][:])
                    nc.tensor.matmul(pOut[:], vh_ones(h, kb)[:], st[:],
                                     start=(i == 0), stop=(i == len(blocks) - 1))
                # grouped stores: blocks are kb=0..nkb-1 in order; split across queues
                ngrp = min(4, nkb)
                bnds = [round(j * nkb / ngrp) for j in range(ngrp + 1)]
                for j in range(ngrp):
                    lo, hi = bnds[j], bnds[j + 1]
                    if hi > lo:
                        nc.sync.dma_start(attnT_d[h, c, :, lo:hi, :],
                                          stc[:, lo:hi, :])
                sumrow = misc.tile([128, 512], F32, tag="sumrow", name="sumrow")
                nc.scalar.activation(sumrow[64:65, :], pOut[DH:DH + 1, :],
                                     mybir.ActivationFunctionType.Copy)
                nc.sync.dma_start(sums_d[h, c], sumrow[64:65, :])
                row0 = misc.tile([1, 512], F32, tag="row0", name="row0")
                nc.sync.dma_start(row0[:], sumrow[64:65, :])
                recip0 = misc.tile([1, 512], F32, tag="recip0", name="recip0")
                nc.vector.reciprocal_approx_fast(recip0[:], row0[:])
                recip_bc = misc.tile([64, 512], F32, tag="recipbc", name="recipbc")
                nc.gpsimd.partition_broadcast(recip_bc[:], recip0[:])
                nc.vector.tensor_mul(aoT[m][po:po + 64, cs], pOut[0:DH, :],
                                     recip_bc[:])

        # ---- phase 4: output projection -----------------------------------------
        for tt in range(NTOK):
            ev = misc.tile([128, 2, 512], F32, tag="oev", name="ev")
            for nn in range(2):
                p = psA.tile([128, 512], F32, tag="psA", name="po")
                for m in range(2):
                    nc.tensor.matmul(p[:], aoT[m][:, tt * 128:(tt + 1) * 128],
                                     wo_s[m][:, nn * 512:(nn + 1) * 512],
                                     start=(m == 0), stop=(m == 1))
                nc.vector.tensor_copy(ev[:, nn, :], p[:])
            nc.sync.dma_start(outp_d[tt], ev[:])

    nc.compile()
    return nc


def _mask_schedule(mask):
    """Classify (k-block 128) x (q-chunk 512) tiles from mask (S,S), 1=masked."""
    masked = mask >= 0.5
    schedule = []
    tiles = []
    tile_index = {}
    for c in range(NCHK):
        blocks = []
        for kb in range(NTOK):
            sub = masked[c * 512:(c + 1) * 512, kb * 128:(kb + 1) * 128]  # (q, k)
            if sub.all():
                continue
            if not sub.any():
                blocks.append((kb, None))
                continue
            t = np.ascontiguousarray((~sub).T.astype(np.float32))  # (k128, q512)
            key = t.tobytes()
            if key not in tile_index:
                tile_index[key] = len(tiles)
                tiles.append(t)
            blocks.append((kb, tile_index[key]))
        schedule.append(blocks)
    return schedule, tiles


def _block_T(x):
    """(S, DM) -> transposed blocked [NCHK, 128, KT, 512] contiguous."""
    # block [cc, p, kt, j] = x[cc*512+j, kt*128+p]
    return np.ascontiguousarray(
        x.T.reshape(KT, 128, NCHK, 512).transpose(2, 1, 0, 3))


def kernel(q, k, v, mask, Wq, bq, Wk, bk, Wv, bv, Wo, bo, _trace=False):
    q = np.asarray(q, np.float32)
    k = np.asarray(k, np.float32)
    v = np.asarray(v, np.float32)
    mask2d = np.asarray(mask, np.float32).reshape(S, S)
    Wq, Wk, Wv, Wo = (np.asarray(x, np.float32) for x in (Wq, Wk, Wv, Wo))
    bq, bk, bv, bo = (np.asarray(x, np.float32) for x in (bq, bk, bv, bo))

    use_f32r = USE_F32R
    ndt = np.float32 if use_f32r else ml_dtypes.bfloat16

    schedule, mtiles = _mask_schedule(mask2d)
    key = (tuple(tuple(b) for b in schedule), len(mtiles), use_f32r)
    if key not in _CACHE:
        _CACHE[key] = _build_program(schedule, len(mtiles), use_f32r)
    nc = _CACHE[key]

    mt_arr = np.stack(mtiles).astype(ndt) if mtiles else None
    in_maps = []
    for c in range(8):
        b, g = c // 4, c % 4
        hs = slice(g * HD, (g + 1) * HD)
        im = {
            "qT": _block_T(q[b]).astype(ndt),
            "kT": _block_T(k[b]).astype(ndt),
            "vT": _block_T(v[b]).astype(ndt),
            "wq": np.ascontiguousarray(
                Wq[:, hs].reshape(KT, 128, HD).transpose(1, 0, 2)).astype(ndt),
            "wk": np.ascontiguousarray(
                Wk[:, hs].reshape(KT, 128, HD).transpose(1, 0, 2)).astype(ndt),
            "wv": np.ascontiguousarray(
                Wv[:, hs].reshape(KT, 128, HD).transpose(1, 0, 2)).astype(ndt),
            "wo": np.ascontiguousarray(Wo[hs, :]).astype(ndt),
            "bq": np.ascontiguousarray(bq[hs].reshape(HD, 1)),
            "bk": np.ascontiguousarray(bk[hs].reshape(HD, 1)),
            "bv": np.ascontiguousarray(bv[hs].reshape(1, HD)),
        }
        if mt_arr is not None:
            im["mt"] = mt_arr
        in_maps.append(im)

    res = bass_utils.run_bass_kernel_spmd(nc, in_maps, core_ids=list(range(8)),
                                          trace=_trace)

    out = np.zeros((B, S, DM), np.float32)
    attn = np.empty((B, NHEADS, S, S), np.float32)
    for c in range(8):
        b, g = c // 4, c % 4
        r = res.results[c]
        op = np.asarray(r["outp"], np.float32)  # [NTOK, 128, 2, 512]
        out[b] += op.reshape(S, DM)
        at = np.asarray(r["attnT"]).astype(np.float32)  # [NH, NCHK, 128, NTOK, 512]
        sums = np.asarray(r["sums"], np.float32).reshape(NH, S)  # per-q row sums
        for h in range(NH):
            # slot i in the NTOK axis holds k-block schedule[c][i][0]
            a = np.zeros((S, S), np.float32)
            for ci in range(NCHK):
                blk = at[h, ci].transpose(1, 0, 2)  # [NTOK, 128(kl), 512(qj)]
                for i, (kb, _mi) in enumerate(schedule[ci]):
                    a[ci * 512:(ci + 1) * 512, kb * 128:(kb + 1) * 128] = blk[i].T
            a /= sums[h][:, None]
            attn[b, g * NH + h] = a
    out += bo
    if _trace:
        kernel.last_results = res
    return out, attn


# revision 14
# speedup vs baseline: 2.2555x; 1.0336x over previous
"""Multi-head attention (B=2, S=2048, D=1024, H=16, causal) on 8 TRN2 NeuronCores.

Sharding: 8 shards = 2 batches x 4 head-groups (4 heads each). Each core:
  - projects q/k/v for its batch through its head-group's weight slices
    (qhT/khT computed transposed: [head_dim, tok]; vh natural: [tok, head_dim])
  - computes causal attention per head in the transposed layout
    ST[k_tok, q_tok] = Kh @ Qh^T, exp (no max-subtraction: logits are O(1) and
    masked entries are never computed), a column of ones appended to the AV
    stationary yields the softmax denominators for free,
  - normalizes, writes attn^T per head (only causal blocks; the rest of the
    output buffer stays zero), in a DMA-contiguous blocked layout,
  - computes its partial output projection out_part = (attn @ vh) @ Wo_slice.
Host: shards/transposes inputs into blocked layouts, sums the 4 partial
outputs per batch, adds bo, and un-blocks/transposes attn back.

Matmul dtype: bf16 (full-rate PE) by default; float32r (half-rate, ~1.6e-4)
via KERNEL_F32R=1. PSUM accumulation is fp32 in both.
"""
import os
import sys

sys.path.insert(0, "/opt/trn_rl_repo")

import numpy as np
import ml_dtypes
from contextlib import ExitStack

import concourse.bass as bass
import concourse.tile as tile
from concourse import bacc, mybir
from concourse import bass_utils

F32 = mybir.dt.float32
F32R = mybir.dt.float32r
BF16 = mybir.dt.bfloat16

B, S, DM, NHEADS = 2, 2048, 1024, 16
NH = 4             # heads per core
DH = DM // NHEADS  # 64
HD = NH * DH       # 256 head dims per core
KT = DM // 128     # 8 contraction tiles for projections
NCHK = S // 512    # 4 q-chunks
NHC = S // 256     # 8 projection half-chunks
NTOK = S // 128    # 16 token tiles / k-blocks
SCALE = 1.0 / np.sqrt(np.float32(DH))

USE_F32R = os.environ.get("KERNEL_F32R", "0") == "1"

_CACHE = {}


def _build_program(schedule, nmask, use_f32r):
    """schedule: per chunk c, list of (kb, mask_idx|None). Same on all cores."""
    DT = F32R if use_f32r else BF16
    nc = bacc.Bacc("TRN2", target_bir_lowering=False, debug=False)

    # blocked inputs: [KT, NHC, 128, 256] so each DMA'd tile is contiguous
    qT_d = nc.dram_tensor("qT", [NCHK, 128, KT, 512], DT, kind="ExternalInput").ap()
    kT_d = nc.dram_tensor("kT", [NCHK, 128, KT, 512], DT, kind="ExternalInput").ap()
    vT_d = nc.dram_tensor("vT", [NCHK, 128, KT, 512], DT, kind="ExternalInput").ap()
    wq_d = nc.dram_tensor("wq", [128, KT, HD], DT, kind="ExternalInput").ap()
    wk_d = nc.dram_tensor("wk", [128, KT, HD], DT, kind="ExternalInput").ap()
    wv_d = nc.dram_tensor("wv", [128, KT, HD], DT, kind="ExternalInput").ap()
    wo_d = nc.dram_tensor("wo", [HD, DM], DT, kind="ExternalInput").ap()
    bq_d = nc.dram_tensor("bq", [HD, 1], F32, kind="ExternalInput").ap()
    bk_d = nc.dram_tensor("bk", [HD, 1], F32, kind="ExternalInput").ap()
    bv_d = nc.dram_tensor("bv", [1, HD], F32, kind="ExternalInput").ap()
    mt_d = None
    if nmask:
        mt_d = nc.dram_tensor("mt", [nmask, 128, 512], DT, kind="ExternalInput").ap()

    # blocked outputs: every [128, 512] store is one contiguous region
    attnT_d = nc.dram_tensor("attnT", [NH, NCHK, 128, NTOK, 512], DT,
                             kind="ExternalOutput").ap()
    outp_d = nc.dram_tensor("outp", [NTOK, 128, 2, 512], F32,
                            kind="ExternalOutput").ap()
    sums_d = nc.dram_tensor("sums", [NH, NCHK, 1, 512], F32,
                            kind="ExternalOutput").ap()

    with tile.TileContext(nc) as tc, ExitStack() as ctx:
        wpool = ctx.enter_context(tc.tile_pool(name="w", bufs=1))
        stream = ctx.enter_context(tc.tile_pool(name="stream", bufs=2 if not use_f32r else 1))
        stexp = ctx.enter_context(tc.tile_pool(name="stexp", bufs=3 if not use_f32r else 1))
        misc = ctx.enter_context(tc.tile_pool(name="misc", bufs=2))
        psA = ctx.enter_context(tc.tile_pool(name="psA", bufs=2, space="PSUM"))
        psS = ctx.enter_context(tc.tile_pool(name="psS", bufs=2, space="PSUM"))
        psO = ctx.enter_context(tc.tile_pool(name="psO", bufs=2, space="PSUM"))

        # ---- persistent constants ------------------------------------------------
        wqkv = {}
        for name, srcd in (("wq", wq_d), ("wk", wk_d), ("wv", wv_d)):
            t = wpool.tile([128, KT, HD], DT, tag=name, name=name)
            nc.sync.dma_start(t[:], srcd)
            wqkv[name] = t
        wq_s = [wqkv["wq"][:, kt, :] for kt in range(KT)]
        wk_s = [wqkv["wk"][:, kt, :] for kt in range(KT)]
        wv_s = [wqkv["wv"][:, kt, :] for kt in range(KT)]
        wo_s = []
        for m in range(2):
            t = wpool.tile([128, DM], DT, tag=f"wo{m}", name=f"wo{m}")
            nc.sync.dma_start(t[:], wo_d[m * 128:(m + 1) * 128, :])
            wo_s.append(t)
        bqt, bkt = [], []
        for m in range(2):
            t = wpool.tile([128, 1], F32, tag=f"bq{m}", name=f"bq{m}")
            nc.sync.dma_start(t[:], bq_d[m * 128:(m + 1) * 128, :])
            bqt.append(t)
            t = wpool.tile([128, 1], F32, tag=f"bk{m}", name=f"bk{m}")
            nc.sync.dma_start(t[:], bk_d[m * 128:(m + 1) * 128, :])
            bkt.append(t)
        bv_row = wpool.tile([1, HD], F32, tag="bvrow")
        nc.sync.dma_start(bv_row[:], bv_d)
        bv_bc = wpool.tile([128, HD], F32, tag="bvbc")
        nc.gpsimd.partition_broadcast(bv_bc[:], bv_row[:])

        mt_s = []
        for i in range(nmask):
            t = wpool.tile([128, 512], DT, tag=f"mt{i}", name=f"mt{i}")
            nc.sync.dma_start(t[:], mt_d[i])
            mt_s.append(t)

        ones_f32 = wpool.tile([128, 1], F32, tag="ones")
        nc.vector.memset(ones_f32[:], 1.0)

        qhT = [wpool.tile([128, S], DT, tag=f"qhT{m}", name=f"qhT{m}")
               for m in range(2)]
        khT = [wpool.tile([128, S], DT, tag=f"khT{m}", name=f"khT{m}")
               for m in range(2)]
        aoT = [wpool.tile([128, S], DT, tag=f"aoT{m}", name=f"aoT{m}")
               for m in range(2)]
        vh_pack = [wpool.tile([128, NTOK * (DH + 1)], DT, tag=f"vhp{h}",
                              name=f"vhp{h}") for h in range(NH)]

        def vh_ones(h, tt):
            return vh_pack[h][:, tt * (DH + 1):(tt + 1) * (DH + 1)]

        for h in range(NH):
            for tt in range(NTOK):
                nc.vector.tensor_copy(vh_ones(h, tt)[:, DH:DH + 1], ones_f32[:])

        # ---- phase 2a: q/k projections (transposed: [head_dim, tok]) ------------
        for cc in range(NCHK):
            cs = slice(cc * 512, (cc + 1) * 512)
            qin = stream.tile([128, KT, 512], DT, tag="qin", name="qin")
            nc.sync.dma_start(qin[:], qT_d[cc])
            kin = stream.tile([128, KT, 512], DT, tag="kin", name="kin")
            nc.sync.dma_start(kin[:], kT_d[cc])
            for m in range(2):
                p = psA.tile([128, 512], F32, tag="psA", name="pq")
                for kt in range(KT):
                    nc.tensor.matmul(p[:], wq_s[kt][:, m * 128:(m + 1) * 128],
                                     qin[:, kt, :], start=(kt == 0),
                                     stop=(kt == KT - 1))
                nc.scalar.activation(qhT[m][:, cs], p[:],
                                     mybir.ActivationFunctionType.Identity,
                                     bias=bqt[m][:])
                p = psA.tile([128, 512], F32, tag="psA", name="pk")
                for kt in range(KT):
                    nc.tensor.matmul(p[:], wk_s[kt][:, m * 128:(m + 1) * 128],
                                     kin[:, kt, :], start=(kt == 0),
                                     stop=(kt == KT - 1))
                nc.scalar.activation(khT[m][:, cs], p[:],
                                     mybir.ActivationFunctionType.Identity,
                                     bias=bkt[m][:])

        # ---- phase 2b: v projection (natural: [tok, head_dim]) ------------------
        for cc in range(NCHK):
            vin = stream.tile([128, KT, 512], DT, tag="qin", name="vin")
            nc.sync.dma_start(vin[:], vT_d[cc])
            for tl in range(4):
                tt = cc * 4 + tl
                p = psA.tile([128, HD], F32, tag="psA", name="pv")
                for kt in range(KT):
                    nc.tensor.matmul(p[:], vin[:, kt, tl * 128:(tl + 1) * 128],
                                     wv_s[kt][:], start=(kt == 0),
                                     stop=(kt == KT - 1))
                pb = misc.tile([128, HD], F32, tag="vbias", name="pb")
                nc.vector.tensor_add(pb[:], p[:], bv_bc[:])
                for h in range(NH):
                    nc.vector.tensor_copy(vh_ones(h, tt)[:, 0:DH],
                                          pb[:, h * DH:(h + 1) * DH])

        # ---- phase 3: attention --------------------------------------------------
        for h in range(NH):
            m, po = h // 2, (h % 2) * 64
            for c in range(NCHK):
                blocks = schedule[c]
                cs = slice(c * 512, (c + 1) * 512)
                pOut = psO.tile([DH + 1, 512], F32, tag="psO", name="pOut")
                nkb = len(blocks)
                stc = stexp.tile([128, NTOK * 512], DT, tag="st", name="stc")
                # process adjacent k-blocks in pairs: one 2-bank PSUM tile and
                # a single 1024-wide exp per pair halves ACT instruction count
                i = 0
                while i < nkb:
                    pair = min(2, nkb - i)
                    pS = psS.tile([128, 1024], F32, tag="psS", name="pS")
                    for u in range(pair):
                        kb = blocks[i + u][0]
                        nc.tensor.matmul(pS[:, u * 512:(u + 1) * 512],
                                         khT[m][po:po + 64,
                                                kb * 128:(kb + 1) * 128],
                                         qhT[m][po:po + 64, cs],
                                         start=True, stop=True)
                    stp = stc[:, i * 512:(i + pair) * 512]
                    nc.scalar.activation(stp[:], pS[:, 0:pair * 512],
                                         mybir.ActivationFunctionType.Exp,
                                         scale=float(SCALE))
                    for u in range(pair):
                        kb, mi = blocks[i + u]
                        st = stc[:, (i + u) * 512:(i + u + 1) * 512]
                        if mi is not None:
                            nc.vector.tensor_mul(st[:], st[:], mt_s[mi][:])
                        nc.tensor.matmul(pOut[:], vh_ones(h, kb)[:], st[:],
                                         start=(i + u == 0),
                                         stop=(i + u == nkb - 1))
                    i += pair
                # grouped stores: split across queues for DMA-engine parallelism
                ngrp = min(4, nkb)
                bnds = [round(j * nkb / ngrp) for j in range(ngrp + 1)]
                for j in range(ngrp):
                    lo, hi = bnds[j], bnds[j + 1]
                    if hi > lo:
                        nc.sync.dma_start(attnT_d[h, c, :, lo:hi, :],
                                          stc[:, lo * 512:hi * 512])
                sumrow = misc.tile([128, 512], F32, tag="sumrow", name="sumrow")
                nc.scalar.activation(sumrow[64:65, :], pOut[DH:DH + 1, :],
                                     mybir.ActivationFunctionType.Copy)
                nc.sync.dma_start(sums_d[h, c], sumrow[64:65, :])
                row0 = misc.tile([1, 512], F32, tag="row0", name="row0")
                nc.sync.dma_start(row0[:], sumrow[64:65, :])
                recip0 = misc.tile([1, 512], F32, tag="recip0", name="recip0")
                nc.vector.reciprocal_approx_fast(recip0[:], row0[:])
                recip_bc = misc.tile([64, 512], F32, tag="recipbc", name="recipbc")
                nc.gpsimd.partition_broadcast(recip_bc[:], recip0[:])
                nc.vector.tensor_mul(aoT[m][po:po + 64, cs], pOut[0:DH, :],
                                     recip_bc[:])

        # ---- phase 4: output projection -----------------------------------------
        for tt in range(NTOK):
            ev = misc.tile([128, 2, 512], F32, tag="oev", name="ev")
            for nn in range(2):
                p = psA.tile([128, 512], F32, tag="psA", name="po")
                for m in range(2):
                    nc.tensor.matmul(p[:], aoT[m][:, tt * 128:(tt + 1) * 128],
                                     wo_s[m][:, nn * 512:(nn + 1) * 512],
                                     start=(m == 0), stop=(m == 1))
                nc.vector.tensor_copy(ev[:, nn, :], p[:])
            nc.sync.dma_start(outp_d[tt], ev[:])

    nc.compile()
    return nc


def _mask_schedule(mask):
    """Classify (k-block 128) x (q-chunk 512) tiles from mask (S,S), 1=masked."""
    masked = mask >= 0.5
    schedule = []
    tiles = []
    tile_index = {}
    for c in range(NCHK):
        blocks = []
        for kb in range(NTOK):
            sub = masked[c * 512:(c + 1) * 512, kb * 128:(kb + 1) * 128]  # (q, k)
            if sub.all():
                continue
            if not sub.any():
                blocks.append((kb, None))
                continue
            t = np.ascontiguousarray((~sub).T.astype(np.float32))  # (k128, q512)
            key = t.tobytes()
            if key not in tile_index:
                tile_index[key] = len(tiles)
                tiles.append(t)
            blocks.append((kb, tile_index[key]))
        schedule.append(blocks)
    return schedule, tiles


def _block_T(x):
    """(S, DM) -> transposed blocked [NCHK, 128, KT, 512] contiguous."""
    # block [cc, p, kt, j] = x[cc*512+j, kt*128+p]
    return np.ascontiguousarray(
        x.T.reshape(KT, 128, NCHK, 512).transpose(2, 1, 0, 3))


def kernel(q, k, v, mask, Wq, bq, Wk, bk, Wv, bv, Wo, bo, _trace=False):
    q = np.asarray(q, np.float32)
    k = np.asarray(k, np.float32)
    v = np.asarray(v, np.float32)
    mask2d = np.asarray(mask, np.float32).reshape(S, S)
    Wq, Wk, Wv, Wo = (np.asarray(x, np.float32) for x in (Wq, Wk, Wv, Wo))
    bq, bk, bv, bo = (np.asarray(x, np.float32) for x in (bq, bk, bv, bo))

    use_f32r = USE_F32R
    ndt = np.float32 if use_f32r else ml_dtypes.bfloat16

    schedule, mtiles = _mask_schedule(mask2d)
    key = (tuple(tuple(b) for b in schedule), len(mtiles), use_f32r)
    if key not in _CACHE:
        _CACHE[key] = _build_program(schedule, len(mtiles), use_f32r)
    nc = _CACHE[key]

    mt_arr = np.stack(mtiles).astype(ndt) if mtiles else None
    in_maps = []
    for c in range(8):
        b, g = c // 4, c % 4
        hs = slice(g * HD, (g + 1) * HD)
        im = {
            "qT": _block_T(q[b]).astype(ndt),
            "kT": _block_T(k[b]).astype(ndt),
            "vT": _block_T(v[b]).astype(ndt),
            "wq": np.ascontiguousarray(
                Wq[:, hs].reshape(KT, 128, HD).transpose(1, 0, 2)).astype(ndt),
            "wk": np.ascontiguousarray(
                Wk[:, hs].reshape(KT, 128, HD).transpose(1, 0, 2)).astype(ndt),
            "wv": np.ascontiguousarray(
                Wv[:, hs].reshape(KT, 128, HD).transpose(1, 0, 2)).astype(ndt),
            "wo": np.ascontiguousarray(Wo[hs, :]).astype(ndt),
            "bq": np.ascontiguousarray(bq[hs].reshape(HD, 1)),
            "bk": np.ascontiguousarray(bk[hs].reshape(HD, 1)),
            "bv": np.ascontiguousarray(bv[hs].reshape(1, HD)),
        }
        if mt_arr is not None:
            im["mt"] = mt_arr
        in_maps.append(im)

    res = bass_utils.run_bass_kernel_spmd(nc, in_maps, core_ids=list(range(8)),
                                          trace=_trace)

    out = np.zeros((B, S, DM), np.float32)
    attn = np.empty((B, NHEADS, S, S), np.float32)
    for c in range(8):
        b, g = c // 4, c % 4
        r = res.results[c]
        op = np.asarray(r["outp"], np.float32)  # [NTOK, 128, 2, 512]
        out[b] += op.reshape(S, DM)
        at = np.asarray(r["attnT"]).astype(np.float32)  # [NH, NCHK, 128, NTOK, 512]
        sums = np.asarray(r["sums"], np.float32).reshape(NH, S)  # per-q row sums
        for h in range(NH):
            # slot i in the NTOK axis holds k-block schedule[c][i][0]
            a = np.zeros((S, S), np.float32)
            for ci in range(NCHK):
                blk = at[h, ci].transpose(1, 0, 2)  # [NTOK, 128(kl), 512(qj)]
                for i, (kb, _mi) in enumerate(schedule[ci]):
                    a[ci * 512:(ci + 1) * 512, kb * 128:(kb + 1) * 128] = blk[i].T
            a /= sums[h][:, None]
            attn[b, g * NH + h] = a
    out += bo
    if _trace:
        kernel.last_results = res
    return out, attn


# revision 15
# speedup vs baseline: 2.3377x; 1.0365x over previous
"""Multi-head attention (B=2, S=2048, D=1024, H=16, causal) on 8 TRN2 NeuronCores.

Sharding: 8 shards = 2 batches x 4 head-groups (4 heads each). Each core:
  - projects q/k/v for its batch through its head-group's weight slices
    (qhT/khT computed transposed: [head_dim, tok]; vh natural: [tok, head_dim])
  - computes causal attention per head in the transposed layout
    ST[k_tok, q_tok] = Kh @ Qh^T, exp (no max-subtraction: logits are O(1) and
    masked entries are never computed), a column of ones appended to the AV
    stationary yields the softmax denominators for free,
  - normalizes, writes attn^T per head (only causal blocks; the rest of the
    output buffer stays zero), in a DMA-contiguous blocked layout,
  - computes its partial output projection out_part = (attn @ vh) @ Wo_slice.
Host: shards/transposes inputs into blocked layouts, sums the 4 partial
outputs per batch, adds bo, and un-blocks/transposes attn back.

Matmul dtype: bf16 (full-rate PE) by default; float32r (half-rate, ~1.6e-4)
via KERNEL_F32R=1. PSUM accumulation is fp32 in both.
"""
import os
import sys

sys.path.insert(0, "/opt/trn_rl_repo")

import numpy as np
import ml_dtypes
from contextlib import ExitStack

import concourse.bass as bass
import concourse.tile as tile
from concourse import bacc, mybir
from concourse import bass_utils

F32 = mybir.dt.float32
F32R = mybir.dt.float32r
BF16 = mybir.dt.bfloat16

B, S, DM, NHEADS = 2, 2048, 1024, 16
NH = 4             # heads per core
DH = DM // NHEADS  # 64
HD = NH * DH       # 256 head dims per core
KT = DM // 128     # 8 contraction tiles for projections
NCHK = S // 512    # 4 q-chunks
NHC = S // 256     # 8 projection half-chunks
NTOK = S // 128    # 16 token tiles / k-blocks
SCALE = 1.0 / np.sqrt(np.float32(DH))

USE_F32R = os.environ.get("KERNEL_F32R", "0") == "1"

_CACHE = {}


def _build_program(schedule, nmask, use_f32r):
    """schedule: per chunk c, list of (kb, mask_idx|None). Same on all cores."""
    DT = F32R if use_f32r else BF16
    nc = bacc.Bacc("TRN2", target_bir_lowering=False, debug=False)

    # blocked inputs: [KT, NHC, 128, 256] so each DMA'd tile is contiguous
    qT_d = nc.dram_tensor("qT", [NCHK, 128, KT, 512], DT, kind="ExternalInput").ap()
    kT_d = nc.dram_tensor("kT", [NCHK, 128, KT, 512], DT, kind="ExternalInput").ap()
    vT_d = nc.dram_tensor("vT", [NCHK, 128, KT, 512], DT, kind="ExternalInput").ap()
    wq_d = nc.dram_tensor("wq", [128, KT, HD], DT, kind="ExternalInput").ap()
    wk_d = nc.dram_tensor("wk", [128, KT, HD], DT, kind="ExternalInput").ap()
    wv_d = nc.dram_tensor("wv", [128, KT, HD], DT, kind="ExternalInput").ap()
    wo_d = nc.dram_tensor("wo", [HD, DM], DT, kind="ExternalInput").ap()
    bq_d = nc.dram_tensor("bq", [HD, 1], F32, kind="ExternalInput").ap()
    bk_d = nc.dram_tensor("bk", [HD, 1], F32, kind="ExternalInput").ap()
    bv_d = nc.dram_tensor("bv", [1, HD], F32, kind="ExternalInput").ap()
    mt_d = None
    if nmask:
        mt_d = nc.dram_tensor("mt", [nmask, 128, 512], DT, kind="ExternalInput").ap()

    # blocked outputs: every [128, 512] store is one contiguous region
    attnT_d = nc.dram_tensor("attnT", [NH, NCHK, 128, NTOK, 512], DT,
                             kind="ExternalOutput").ap()
    outp_d = nc.dram_tensor("outp", [NTOK, 128, 2, 512], F32,
                            kind="ExternalOutput").ap()
    sums_d = nc.dram_tensor("sums", [NH, NCHK, 1, 512], F32,
                            kind="ExternalOutput").ap()

    with tile.TileContext(nc) as tc, ExitStack() as ctx:
        wpool = ctx.enter_context(tc.tile_pool(name="w", bufs=1))
        stream = ctx.enter_context(tc.tile_pool(name="stream", bufs=2 if not use_f32r else 1))
        stexp = ctx.enter_context(tc.tile_pool(name="stexp", bufs=3 if not use_f32r else 1))
        misc = ctx.enter_context(tc.tile_pool(name="misc", bufs=2))
        psA = ctx.enter_context(tc.tile_pool(name="psA", bufs=2, space="PSUM"))
        psS = ctx.enter_context(tc.tile_pool(name="psS", bufs=2, space="PSUM"))
        psO = ctx.enter_context(tc.tile_pool(name="psO", bufs=2, space="PSUM"))

        # ---- persistent constants ------------------------------------------------
        wqkv = {}
        for name, srcd in (("wq", wq_d), ("wk", wk_d), ("wv", wv_d)):
            t = wpool.tile([128, KT, HD], DT, tag=name, name=name)
            nc.sync.dma_start(t[:], srcd)
            wqkv[name] = t
        wq_s = [wqkv["wq"][:, kt, :] for kt in range(KT)]
        wk_s = [wqkv["wk"][:, kt, :] for kt in range(KT)]
        wv_s = [wqkv["wv"][:, kt, :] for kt in range(KT)]
        wo_s = []
        for m in range(2):
            t = wpool.tile([128, DM], DT, tag=f"wo{m}", name=f"wo{m}")
            nc.sync.dma_start(t[:], wo_d[m * 128:(m + 1) * 128, :])
            wo_s.append(t)
        bqt, bkt = [], []
        for m in range(2):
            t = wpool.tile([128, 1], F32, tag=f"bq{m}", name=f"bq{m}")
            nc.sync.dma_start(t[:], bq_d[m * 128:(m + 1) * 128, :])
            bqt.append(t)
            t = wpool.tile([128, 1], F32, tag=f"bk{m}", name=f"bk{m}")
            nc.sync.dma_start(t[:], bk_d[m * 128:(m + 1) * 128, :])
            bkt.append(t)
        bv_row = wpool.tile([1, HD], F32, tag="bvrow")
        nc.sync.dma_start(bv_row[:], bv_d)
        bv_bc = wpool.tile([128, HD], F32, tag="bvbc")
        nc.gpsimd.partition_broadcast(bv_bc[:], bv_row[:])

        mt_s = []
        for i in range(nmask):
            t = wpool.tile([128, 512], DT, tag=f"mt{i}", name=f"mt{i}")
            nc.sync.dma_start(t[:], mt_d[i])
            mt_s.append(t)

        ones_f32 = wpool.tile([128, 1], F32, tag="ones")
        nc.vector.memset(ones_f32[:], 1.0)

        qhT = [wpool.tile([128, S], DT, tag=f"qhT{m}", name=f"qhT{m}")
               for m in range(2)]
        khT = [wpool.tile([128, S], DT, tag=f"khT{m}", name=f"khT{m}")
               for m in range(2)]
        aoT = [wpool.tile([128, S], DT, tag=f"aoT{m}", name=f"aoT{m}")
               for m in range(2)]
        vh_pack = [wpool.tile([128, NTOK * (DH + 1)], DT, tag=f"vhp{h}",
                              name=f"vhp{h}") for h in range(NH)]

        def vh_ones(h, tt):
            return vh_pack[h][:, tt * (DH + 1):(tt + 1) * (DH + 1)]

        for h in range(NH):
            for tt in range(NTOK):
                nc.vector.tensor_copy(vh_ones(h, tt)[:, DH:DH + 1], ones_f32[:])

        # ---- phase 2a: q/k projections (transposed: [head_dim, tok]) ------------
        for cc in range(NCHK):
            cs = slice(cc * 512, (cc + 1) * 512)
            qin = stream.tile([128, KT, 512], DT, tag="qin", name="qin")
            nc.sync.dma_start(qin[:], qT_d[cc])
            kin = stream.tile([128, KT, 512], DT, tag="kin", name="kin")
            nc.sync.dma_start(kin[:], kT_d[cc])
            for m in range(2):
                p = psA.tile([128, 512], F32, tag="psA", name="pq")
                for kt in range(KT):
                    nc.tensor.matmul(p[:], wq_s[kt][:, m * 128:(m + 1) * 128],
                                     qin[:, kt, :], start=(kt == 0),
                                     stop=(kt == KT - 1))
                nc.scalar.activation(qhT[m][:, cs], p[:],
                                     mybir.ActivationFunctionType.Identity,
                                     bias=bqt[m][:])
                p = psA.tile([128, 512], F32, tag="psA", name="pk")
                for kt in range(KT):
                    nc.tensor.matmul(p[:], wk_s[kt][:, m * 128:(m + 1) * 128],
                                     kin[:, kt, :], start=(kt == 0),
                                     stop=(kt == KT - 1))
                nc.scalar.activation(khT[m][:, cs], p[:],
                                     mybir.ActivationFunctionType.Identity,
                                     bias=bkt[m][:])

        # ---- phase 2b: v projection (natural: [tok, head_dim]) ------------------
        for cc in range(NCHK):
            vin = stream.tile([128, KT, 512], DT, tag="qin", name="vin")
            nc.sync.dma_start(vin[:], vT_d[cc])
            for tl in range(4):
                tt = cc * 4 + tl
                p = psA.tile([128, HD], F32, tag="psA", name="pv")
                for kt in range(KT):
                    nc.tensor.matmul(p[:], vin[:, kt, tl * 128:(tl + 1) * 128],
                                     wv_s[kt][:], start=(kt == 0),
                                     stop=(kt == KT - 1))
                pb = misc.tile([128, HD], F32, tag="vbias", name="pb")
                nc.vector.tensor_add(pb[:], p[:], bv_bc[:])
                for h in range(NH):
                    nc.vector.tensor_copy(vh_ones(h, tt)[:, 0:DH],
                                          pb[:, h * DH:(h + 1) * DH])

        # ---- phase 3: attention --------------------------------------------------
        for c in range(NCHK):
            for h in range(NH):
                m, po = h // 2, (h % 2) * 64
                blocks = schedule[c]
                cs = slice(c * 512, (c + 1) * 512)
                pOut = psO.tile([DH + 1, 512], F32, tag="psO", name="pOut")
                nkb = len(blocks)
                stc = stexp.tile([128, NTOK * 512], DT, tag="st", name="stc")
                # process adjacent k-blocks in pairs: one 2-bank PSUM tile and
                # a single 1024-wide exp per pair halves ACT instruction count
                i = 0
                while i < nkb:
                    pair = min(2, nkb - i)
                    pS = psS.tile([128, 1024], F32, tag="psS", name="pS")
                    for u in range(pair):
                        kb = blocks[i + u][0]
                        nc.tensor.matmul(pS[:, u * 512:(u + 1) * 512],
                                         khT[m][po:po + 64,
                                                kb * 128:(kb + 1) * 128],
                                         qhT[m][po:po + 64, cs],
                                         start=True, stop=True)
                    stp = stc[:, i * 512:(i + pair) * 512]
                    nc.scalar.activation(stp[:], pS[:, 0:pair * 512],
                                         mybir.ActivationFunctionType.Exp,
                                         scale=float(SCALE))
                    for u in range(pair):
                        kb, mi = blocks[i + u]
                        st = stc[:, (i + u) * 512:(i + u + 1) * 512]
                        if mi is not None:
                            nc.vector.tensor_mul(st[:], st[:], mt_s[mi][:])
                        nc.tensor.matmul(pOut[:], vh_ones(h, kb)[:], st[:],
                                         start=(i + u == 0),
                                         stop=(i + u == nkb - 1))
                    i += pair
                # grouped stores: split across queues for DMA-engine parallelism
                ngrp = min(4, nkb)
                bnds = [round(j * nkb / ngrp) for j in range(ngrp + 1)]
                for j in range(ngrp):
                    lo, hi = bnds[j], bnds[j + 1]
                    if hi > lo:
                        nc.sync.dma_start(attnT_d[h, c, :, lo:hi, :],
                                          stc[:, lo * 512:hi * 512])
                sumrow = misc.tile([128, 512], F32, tag="sumrow", name="sumrow")
                nc.scalar.activation(sumrow[64:65, :], pOut[DH:DH + 1, :],
                                     mybir.ActivationFunctionType.Copy)
                nc.sync.dma_start(sums_d[h, c], sumrow[64:65, :])
                row0 = misc.tile([1, 512], F32, tag="row0", name="row0")
                nc.sync.dma_start(row0[:], sumrow[64:65, :])
                recip0 = misc.tile([1, 512], F32, tag="recip0", name="recip0")
                nc.vector.reciprocal_approx_fast(recip0[:], row0[:])
                recip_bc = misc.tile([64, 512], F32, tag="recipbc", name="recipbc")
                nc.gpsimd.partition_broadcast(recip_bc[:], recip0[:])
                nc.vector.tensor_mul(aoT[m][po:po + 64, cs], pOut[0:DH, :],
                                     recip_bc[:])

            # ---- output projection for this chunk's tokens (all heads done) ----
            for tt in range(c * 4, (c + 1) * 4):
                ev = misc.tile([128, 2, 512], F32, tag="oev", name="ev")
                for nn in range(2):
                    p = psA.tile([128, 512], F32, tag="psA", name="po")
                    for m in range(2):
                        nc.tensor.matmul(p[:], aoT[m][:, tt * 128:(tt + 1) * 128],
                                         wo_s[m][:, nn * 512:(nn + 1) * 512],
                                         start=(m == 0), stop=(m == 1))
                    nc.vector.tensor_copy(ev[:, nn, :], p[:])
                nc.sync.dma_start(outp_d[tt], ev[:])

    nc.compile()
    return nc


def _mask_schedule(mask):
    """Classify (k-block 128) x (q-chunk 512) tiles from mask (S,S), 1=masked."""
    masked = mask >= 0.5
    schedule = []
    tiles = []
    tile_index = {}
    for c in range(NCHK):
        blocks = []
        for kb in range(NTOK):
            sub = masked[c * 512:(c + 1) * 512, kb * 128:(kb + 1) * 128]  # (q, k)
            if sub.all():
                continue
            if not sub.any():
                blocks.append((kb, None))
                continue
            t = np.ascontiguousarray((~sub).T.astype(np.float32))  # (k128, q512)
            key = t.tobytes()
            if key not in tile_index:
                tile_index[key] = len(tiles)
                tiles.append(t)
            blocks.append((kb, tile_index[key]))
        schedule.append(blocks)
    return schedule, tiles


def _block_T(x):
    """(S, DM) -> transposed blocked [NCHK, 128, KT, 512] contiguous."""
    # block [cc, p, kt, j] = x[cc*512+j, kt*128+p]
    return np.ascontiguousarray(
        x.T.reshape(KT, 128, NCHK, 512).transpose(2, 1, 0, 3))


def kernel(q, k, v, mask, Wq, bq, Wk, bk, Wv, bv, Wo, bo, _trace=False):
    q = np.asarray(q, np.float32)
    k = np.asarray(k, np.float32)
    v = np.asarray(v, np.float32)
    mask2d = np.asarray(mask, np.float32).reshape(S, S)
    Wq, Wk, Wv, Wo = (np.asarray(x, np.float32) for x in (Wq, Wk, Wv, Wo))
    bq, bk, bv, bo = (np.asarray(x, np.float32) for x in (bq, bk, bv, bo))

    use_f32r = USE_F32R
    ndt = np.float32 if use_f32r else ml_dtypes.bfloat16

    schedule, mtiles = _mask_schedule(mask2d)
    key = (tuple(tuple(b) for b in schedule), len(mtiles), use_f32r)
    if key not in _CACHE:
        _CACHE[key] = _build_program(schedule, len(mtiles), use_f32r)
    nc = _CACHE[key]

    mt_arr = np.stack(mtiles).astype(ndt) if mtiles else None
    in_maps = []
    for c in range(8):
        b, g = c // 4, c % 4
        hs = slice(g * HD, (g + 1) * HD)
        im = {
            "qT": _block_T(q[b]).astype(ndt),
            "kT": _block_T(k[b]).astype(ndt),
            "vT": _block_T(v[b]).astype(ndt),
            "wq": np.ascontiguousarray(
                Wq[:, hs].reshape(KT, 128, HD).transpose(1, 0, 2)).astype(ndt),
            "wk": np.ascontiguousarray(
                Wk[:, hs].reshape(KT, 128, HD).transpose(1, 0, 2)).astype(ndt),
            "wv": np.ascontiguousarray(
                Wv[:, hs].reshape(KT, 128, HD).transpose(1, 0, 2)).astype(ndt),
            "wo": np.ascontiguousarray(Wo[hs, :]).astype(ndt),
            "bq": np.ascontiguousarray(bq[hs].reshape(HD, 1)),
            "bk": np.ascontiguousarray(bk[hs].reshape(HD, 1)),
            "bv": np.ascontiguousarray(bv[hs].reshape(1, HD)),
        }
        if mt_arr is not None:
            im["mt"] = mt_arr
        in_maps.append(im)

    res = bass_utils.run_bass_kernel_spmd(nc, in_maps, core_ids=list(range(8)),
                                          trace=_trace)

    out = np.zeros((B, S, DM), np.float32)
    attn = np.empty((B, NHEADS, S, S), np.float32)
    for c in range(8):
        b, g = c // 4, c % 4
        r = res.results[c]
        op = np.asarray(r["outp"], np.float32)  # [NTOK, 128, 2, 512]
        out[b] += op.reshape(S, DM)
        at = np.asarray(r["attnT"]).astype(np.float32)  # [NH, NCHK, 128, NTOK, 512]
        sums = np.asarray(r["sums"], np.float32).reshape(NH, S)  # per-q row sums
        for h in range(NH):
            # slot i in the NTOK axis holds k-block schedule[c][i][0]
            a = np.zeros((S, S), np.float32)
            for ci in range(NCHK):
                blk = at[h, ci].transpose(1, 0, 2)  # [NTOK, 128(kl), 512(qj)]
                for i, (kb, _mi) in enumerate(schedule[ci]):
                    a[ci * 512:(ci + 1) * 512, kb * 128:(kb + 1) * 128] = blk[i].T
            a /= sums[h][:, None]
            attn[b, g * NH + h] = a
    out += bo
    if _trace:
        kernel.last_results = res
    return out, attn
